# revision 6
# baseline (speedup 1.0000x reference)
"""Trainium2 Bass kernel for nn_DirectDepthMapper (histogram_binning).

Pipeline (matches reference.py):
  depth (H,W) -> per-pixel point (px,py,pz) -> pose transform -> masks ->
  (r,c) = round(g{z,x}/0.1 + 200) -> 400x400 histogram of valid points.

Strategy:
  - Scatter-add reformulated as windowed one-hot construction (DVE
    is_equal against iota rows, masked points pushed out of the window by
    arithmetic penalties) contracted on the TensorEngine:
    hist_win = sum_blocks ohR^T @ ohC accumulated in PSUM.
  - The active window (bounding box of reachable bins) is derived on the
    host from a clamped min/max of depth (cheap numpy) + interval
    arithmetic over the pose coefficients; row-tiles that cannot pass the
    height-band mask are skipped and the rest are balanced over 8 cores.
  - Depth ships as affine-quantized u16 (dequantized on device); all
    replicated constants (per-column coefficient rows, per-supergroup
    column iotas) ship as single rows and are partition-broadcast on
    device, so per-core transfer is ~0.5MB.
  - One SPMD call; each core emits its partial window histogram; the host
    sums 8 small windows into the 400x400 output.

Self-contained: hardcodes H=W=2048, 8 cores.
"""
import hashlib
import math
import os
import time as _time

import numpy as np

import jax

# Persistent compilation cache: lets warm calls (and fresh processes on the
# same machine) skip the client-side BIR->NEFF recompile entirely.
try:
    jax.config.update("jax_compilation_cache_dir",
                      os.environ.get("BASS_JAX_CACHE_DIR", "/tmp/bass_jax_cache"))
    jax.config.update("jax_persistent_cache_min_compile_time_secs", 0.0)
    jax.config.update("jax_persistent_cache_min_entry_size_bytes", 0)
except Exception:
    pass

import concourse.bass as bass
import concourse.bacc as bacc
import concourse.mybir as mybir
import concourse.tile as tile
from concourse.bass_interp import get_hw_module
from concourse.bass_utils import run_bass_kernel_spmd

# In-process memo of the HLO->NEFF compile hook (pure function of the HLO
# bytes) as insurance for when the persistent cache misses.
try:
    import libneuronxla
    from concourse import bass2jax as _b2j
    _b2j.install_neuronx_cc_hook()
    if not getattr(libneuronxla, "_bass_memo_cc", None):
        _inner_cc = libneuronxla.neuronx_cc
        _cc_memo = {}

        def _memo_cc(code, code_format, platform_version, file_prefix):
            key = (hashlib.sha256(code).digest(), bytes(code_format),
                   str(platform_version))
            if key not in _cc_memo:
                _cc_memo[key] = _inner_cc(code, code_format, platform_version,
                                          file_prefix)
            return _cc_memo[key]

        libneuronxla.neuronx_cc = _memo_cc
        libneuronxla._bass_memo_cc = True
        _b2j.install_neuronx_cc_hook = lambda: None
except Exception:
    pass

# ---------------- problem constants (from reference.py) ----------------
H = W = 2048
N_CORES = 8
NEAR_TH = np.float32(0.1)
FAR_TH = np.float32(4.0)
H_MIN = np.float32(0.0)
H_MAX = np.float32(1.0)
CAMERA_HEIGHT = np.float32(0.0)
CELLS = int(math.ceil(40.0 / 0.1)) + 1   # 401
M = CELLS - 1                            # 400
SHIFT = math.floor(CELLS / 2.0)          # 200
MIN_PTS = 10

FX = np.float32(W / 2.0)
FY = np.float32(H / 2.0)
CX = int(FX) - 1
CY = int(FY) - 1

MAGIC = np.float32(1.5 * 2**23)          # fp32 round-to-nearest-int trick
CLAMP = 4.25                             # quantization clamp (> FAR_TH)
QLEV = 65535.0

# set by test harness for profiling; kernel() stores wall times here
TRACE = False
LAST_EXEC_NS = {}
P = 128                                  # partitions
ROW_TILES = H // P                       # 16
F32 = mybir.dt.float32
F16 = mybir.dt.float16
U16 = mybir.dt.uint16

_dt = np.float32


def _sxv():
    return ((np.arange(W, dtype=np.float64) - CX) / np.float64(FX)).astype(_dt)


def _syv():
    return ((np.arange(H, dtype=np.float64) - CY) / np.float64(FY)).astype(_dt)


# =====================================================================
# host-side interval arithmetic (plan the bin window + active tiles)
# =====================================================================
def _imul(a, b):
    c = [a[0] * b[0], a[0] * b[1], a[1] * b[0], a[1] * b[1]]
    return (min(c), max(c))


def _iadd(a, b):
    return (a[0] + b[0], a[1] + b[1])


def _coef_rows(pose, row):
    """a_i = pose[row,0]*sxv_i + pose[row,2]; b_j = pose[row,1]*syv_j"""
    p = np.asarray(pose, _dt)
    a = (p[row, 0] * _sxv() + p[row, 2]).astype(_dt)
    b = (p[row, 1] * _syv()).astype(_dt)
    k = float(p[row, 3])
    return a, b, k


def _valid_d(dlo, dhi):
    """hull of [dlo,dhi] restricted to the mask1-valid set |d| in [0.1, 4]."""
    lo, hi = None, None
    for a, b in ((-float(FAR_TH), -float(NEAR_TH)), (float(NEAR_TH), float(FAR_TH))):
        s, e = max(a, dlo), min(b, dhi)
        if s <= e:
            lo = s if lo is None else min(lo, s)
            hi = e if hi is None else max(hi, e)
    if lo is None:
        return None
    return (lo, hi)


def _plan(pose, dlo, dhi):
    d_int = _valid_d(dlo, dhi)
    if d_int is None:
        return None
    ax, bx, kx = _coef_rows(pose, 0)   # gx
    ay, by, ky = _coef_rows(pose, 1)   # gy raw
    az, bz, kz = _coef_rows(pose, 2)   # gz

    def box_for(a, b, k):
        c_int = _iadd((float(a.min()), float(a.max())),
                      (float(b.min()), float(b.max())))
        g = _iadd(_imul(d_int, c_int), (k, k))
        v = (10.0 * g[0] + SHIFT, 10.0 * g[1] + SHIFT)
        lo = int(np.floor(v[0])) - 1
        hi = int(np.ceil(v[1])) + 1
        return max(lo, -1), min(hi, M)

    rbox = box_for(az, bz, kz)
    cbox = box_for(ax, bx, kx)
    if rbox[0] > rbox[1] or cbox[0] > cbox[1]:
        return None

    u_hi = float(CAMERA_HEIGHT - ky - H_MIN)   # valid iff u_lo < w < u_hi
    u_lo = float(CAMERA_HEIGHT - ky - H_MAX)
    a_int = (float(ay.min()), float(ay.max()))
    active = []
    for t in range(ROW_TILES):
        bt = by[t * P:(t + 1) * P]
        c_int = _iadd(a_int, (float(bt.min()), float(bt.max())))
        w_int = _imul(d_int, c_int)
        if w_int[0] < u_hi and w_int[1] > u_lo:
            active.append(t)
    return dict(rbox=rbox, cbox=cbox, active=active,
                ax=ax, bx=bx, kx=kx, ay=ay, by=by, ky=ky,
                az=az, bz=bz, kz=kz, u_lo=u_lo, u_hi=u_hi)


def _pad_to(x, m):
    return ((x + m - 1) // m) * m


def _chunks(lo, hi, cap):
    out = []
    x = lo
    while x <= hi:
        wdt = min(cap, hi - x + 1)
        out.append((x, wdt))
        x += wdt
    return out


# =====================================================================
# phase 1 kernel builder
# =====================================================================
_phase1_cache = {}


def _build_phase1(cfg):
    key = cfg["key"]
    if key in _phase1_cache:
        return _phase1_cache[key]

    n_t = cfg["n_t"]
    nb = cfg["nb"]
    r_chunks = cfg["r_chunks"]      # list of (r0, Wr)
    c_chunks = cfg["c_chunks"]      # list of (c0, Wc)
    ax_const = cfg["ax_const"]      # float or None
    az_const = cfg["az_const"]
    bx_zero = cfg["bx_zero"]
    bz_zero = cfg["bz_zero"]
    ay_zero = cfg["ay_zero"]
    kx = cfg["kx"]
    kz = cfg["kz"]
    u_lo = cfg["u_lo"]
    u_hi = cfg["u_hi"]
    sgc = cfg.get("sgc")          # per-supergroup c windows: (Wcol, bases)

    nc = bacc.Bacc("TRN2", target_bir_lowering=False, debug=False,
                   num_devices=N_CORES)
    d_dram = nc.dram_tensor("d1", [n_t * P, W], U16, kind="ExternalInput").ap()
    # per-row (partition) coefficient columns, packed [P, 4*n_t + 2]
    # (bx, by, bz, 0) per tile, then (qscale, qlo)
    b_dram = nc.dram_tensor("bcols", [P, 4 * n_t + 2], F32,
                            kind="ExternalInput").ap()
    need_ax = ax_const is None
    need_az = az_const is None
    need_ay = not ay_zero
    if need_ax:
        ax_dram = nc.dram_tensor("axr", [1, W], F32, kind="ExternalInput").ap()
    if need_az:
        az_dram = nc.dram_tensor("azr", [1, W], F32, kind="ExternalInput").ap()
    if need_ay:
        ay_dram = nc.dram_tensor("ayr", [1, W], F32, kind="ExternalInput").ap()
    iota_r_dram = {}
    iota_c_dram = {}
    sel_dram = {}
    win_dram = {}
    for ri, (r0, Wr) in enumerate(r_chunks):
        iota_r_dram[ri] = nc.dram_tensor(f"ior{ri}", [1, Wr], F16,
                                         kind="ExternalInput").ap()
        sel_dram[ri] = nc.dram_tensor(f"sel{ri}", [nb * Wr, Wr], F32,
                                      kind="ExternalInput").ap()
    if sgc is None:
        for ci, (c0, Wc) in enumerate(c_chunks):
            iota_c_dram[ci] = nc.dram_tensor(f"ioc{ci}", [1, Wc], F16,
                                             kind="ExternalInput").ap()
    else:
        WCOL = sgc["Wcol"]
        n_super_all = W // nb
        iocf_dram = nc.dram_tensor("iocf", [1, n_super_all * WCOL], F16,
                                   kind="ExternalInput").ap()
    # block-diagonal row mask [nb*Wr, nb]: m8[p, n] = 1 iff p // Wr == n
    m8_dram = {}
    for ri, (r0, Wr) in enumerate(r_chunks):
        m8_dram[ri] = nc.dram_tensor(f"m8_{ri}", [nb * Wr, nb], F32,
                                     kind="ExternalInput").ap()
    for ri, (r0, Wr) in enumerate(r_chunks):
        for ci, (c0, Wc) in enumerate(c_chunks):
            win_dram[(ri, ci)] = nc.dram_tensor(
                f"win{ri}_{ci}", [Wr, Wc], F32, kind="ExternalOutput").ap()

    A = mybir.AluOpType
    SENT_LO = float(min(r0 for r0, _ in r_chunks) - 5)
    SENT_HI = float(max(r0 + w for r0, w in r_chunks) + 4)
    PEN = 256.0  # > sentinel span (Wr+9 <= 137), 4*PEN + |SENT| < 2048 (f16 int-exact)

    with tile.TileContext(nc) as tc:
        with tc.tile_pool(name="const", bufs=1) as cpool, \
             tc.tile_pool(name="sbuf", bufs=2) as pool, \
             tc.tile_pool(name="oh", bufs=2) as ohpool, \
             tc.tile_pool(name="psum", bufs=1, space="PSUM") as psum_pool, \
             tc.tile_pool(name="psum2", bufs=2, space="PSUM") as psum2_pool:

            # ---- constants ----
            ior = {}
            ioc = {}
            sel = {}
            m8 = {}
            for ri, (r0, Wr) in enumerate(r_chunks):
                ior1 = cpool.tile([1, Wr], F16, tag=f"ior1{ri}", name=f"ior1{ri}")
                nc.sync.dma_start(out=ior1, in_=iota_r_dram[ri])
                ior[ri] = cpool.tile([P, Wr], F16, tag=f"ior{ri}", name=f"ior{ri}")
                nc.gpsimd.partition_broadcast(ior[ri], ior1)
                sel[ri] = cpool.tile([nb * Wr, Wr], F32, tag=f"sel{ri}",
                                     name=f"sel{ri}")
                nc.sync.dma_start(out=sel[ri], in_=sel_dram[ri])
                m8[ri] = cpool.tile([nb * Wr, nb], F32, tag=f"m8_{ri}",
                                    name=f"m8_{ri}")
                nc.sync.dma_start(out=m8[ri], in_=m8_dram[ri])
            if sgc is None:
                for ci, (c0, Wc) in enumerate(c_chunks):
                    ioc1 = cpool.tile([1, Wc], F16, tag=f"ioc1{ci}",
                                      name=f"ioc1{ci}")
                    nc.sync.dma_start(out=ioc1, in_=iota_c_dram[ci])
                    ioc[ci] = cpool.tile([P, Wc], F16, tag=f"ioc{ci}",
                                         name=f"ioc{ci}")
                    nc.gpsimd.partition_broadcast(ioc[ci], ioc1)
            else:
                WCOL = sgc["Wcol"]
                n_super_all = W // nb
                iocf1 = cpool.tile([1, n_super_all * WCOL], F16, tag="iocf1")
                nc.sync.dma_start(out=iocf1, in_=iocf_dram)
                iocf = cpool.tile([P, n_super_all * WCOL], F16, tag="iocf")
                nc.gpsimd.partition_broadcast(iocf, iocf1)
                zlh = cpool.tile([P, nb * r_chunks[0][1]], F16, tag="zlh")
                nc.vector.memset(zlh, 0.0)
                zrh = cpool.tile([P, nb * c_chunks[0][1]], F16, tag="zrh")
                nc.vector.memset(zrh, 0.0)
            if need_ax:
                ax1 = cpool.tile([1, W], F32, tag="ax1")
                nc.sync.dma_start(out=ax1, in_=ax_dram)
                ax_t = cpool.tile([P, W], F32, tag="ax")
                nc.gpsimd.partition_broadcast(ax_t, ax1)
            if need_az:
                az1 = cpool.tile([1, W], F32, tag="az1")
                nc.sync.dma_start(out=az1, in_=az_dram)
                az_t = cpool.tile([P, W], F32, tag="az")
                nc.gpsimd.partition_broadcast(az_t, az1)
            if need_ay:
                ay1 = cpool.tile([1, W], F32, tag="ay1")
                nc.sync.dma_start(out=ay1, in_=ay_dram)
                ay_t = cpool.tile([P, W], F32, tag="ay")
                nc.gpsimd.partition_broadcast(ay_t, ay1)
            bcols = cpool.tile([P, 4 * n_t + 2], F32, tag="bcols")
            nc.sync.dma_start(out=bcols, in_=b_dram)
            qs_ap = bcols[:, 4 * n_t + 0:4 * n_t + 1]
            ql_ap = bcols[:, 4 * n_t + 1:4 * n_t + 2]

            psum = {}
            for ri, (r0, Wr) in enumerate(r_chunks):
                for ci, (c0, Wc) in enumerate(c_chunks):
                    psum[(ri, ci)] = psum_pool.tile([nb * Wr, nb * Wc], F32,
                                                    tag=f"ps{ri}_{ci}",
                                                    name=f"ps{ri}_{ci}")

            n_super = W // nb
            if sgc is not None:
                for ri, (r0, Wr) in enumerate(r_chunks):
                    nc.tensor.matmul(psum[(ri, 0)], zlh, zrh,
                                     start=True, stop=False)
            CH = 1024                     # column chunk for pipelining
            n_cc = W // CH
            sg_per_cc = CH // nb
            for t in range(n_t):
                q = pool.tile([P, W], U16, tag="q")
                nc.sync.dma_start(out=q, in_=d_dram[t * P:(t + 1) * P, :])
                # dequant: d = qscale*q + qlo
                d = pool.tile([P, W], F32, tag="d")
                nc.scalar.activation(out=d, in_=q,
                                     func=mybir.ActivationFunctionType.Copy,
                                     bias=0.0, scale=qs_ap)
                nc.vector.tensor_scalar(out=d, in0=d, scalar1=ql_ap,
                                        scalar2=None, op0=A.add)
                bx_ap = bcols[:, 4 * t + 0:4 * t + 1]
                by_ap = bcols[:, 4 * t + 1:4 * t + 2]
                bz_ap = bcols[:, 4 * t + 2:4 * t + 3]

                for cc in range(n_cc):
                    csl = slice(cc * CH, (cc + 1) * CH)
                    dC = d[:, csl]

                    # ---- c index ----
                    vc = pool.tile([P, CH], F32, tag="vc")
                    if ax_const is None:
                        tC = pool.tile([P, CH], F32, tag="tC")
                        nc.vector.tensor_tensor(out=tC, in0=dC,
                                                in1=ax_t[:, csl], op=A.mult)
                        if not bx_zero:
                            nc.vector.scalar_tensor_tensor(
                                out=tC, in0=dC, scalar=bx_ap, in1=tC,
                                op0=A.mult, op1=A.add)
                        nc.vector.tensor_scalar(
                            out=vc, in0=tC, scalar1=10.0,
                            scalar2=float(SHIFT + 10.0 * kx),
                            op0=A.mult, op1=A.add)
                    else:
                        if not bx_zero:
                            tC = pool.tile([P, CH], F32, tag="tC")
                            nc.vector.tensor_scalar(out=tC, in0=dC, scalar1=bx_ap,
                                                    scalar2=None, op0=A.mult)
                            nc.vector.scalar_tensor_tensor(
                                out=tC, in0=dC, scalar=float(ax_const), in1=tC,
                                op0=A.mult, op1=A.add)
                            nc.vector.tensor_scalar(
                                out=vc, in0=tC, scalar1=10.0,
                                scalar2=float(SHIFT + 10.0 * kx),
                                op0=A.mult, op1=A.add)
                        else:
                            nc.vector.tensor_scalar(
                                out=vc, in0=dC, scalar1=float(10.0 * ax_const),
                                scalar2=float(SHIFT + 10.0 * kx),
                                op0=A.mult, op1=A.add)
                    vcM = pool.tile([P, CH], F32, tag="vcM")
                    nc.scalar.activation(out=vcM, in_=vc,
                                         func=mybir.ActivationFunctionType.Copy,
                                         bias=float(MAGIC))
                    vc16 = pool.tile([P, CH], F16, tag="vc16")
                    nc.scalar.activation(out=vc16, in_=vcM,
                                         func=mybir.ActivationFunctionType.Copy,
                                         bias=float(-MAGIC))

                    # ---- r index ----
                    vr = pool.tile([P, CH], F32, tag="vr")
                    if az_const is None:
                        tZ = pool.tile([P, CH], F32, tag="tZ")
                        nc.vector.tensor_tensor(out=tZ, in0=dC,
                                                in1=az_t[:, csl], op=A.mult)
                        if not bz_zero:
                            nc.vector.scalar_tensor_tensor(
                                out=tZ, in0=dC, scalar=bz_ap, in1=tZ,
                                op0=A.mult, op1=A.add)
                        nc.vector.tensor_scalar(
                            out=vr, in0=tZ, scalar1=10.0,
                            scalar2=float(SHIFT + 10.0 * kz),
                            op0=A.mult, op1=A.add)
                    else:
                        if not bz_zero:
                            tZ = pool.tile([P, CH], F32, tag="tZ")
                            nc.vector.tensor_scalar(out=tZ, in0=dC, scalar1=bz_ap,
                                                    scalar2=None, op0=A.mult)
                            nc.vector.scalar_tensor_tensor(
                                out=tZ, in0=dC, scalar=float(az_const), in1=tZ,
                                op0=A.mult, op1=A.add)
                            nc.vector.tensor_scalar(
                                out=vr, in0=tZ, scalar1=10.0,
                                scalar2=float(SHIFT + 10.0 * kz),
                                op0=A.mult, op1=A.add)
                        else:
                            nc.vector.tensor_scalar(
                                out=vr, in0=dC, scalar1=float(10.0 * az_const),
                                scalar2=float(SHIFT + 10.0 * kz),
                                op0=A.mult, op1=A.add)
                    vrM = pool.tile([P, CH], F32, tag="vrM")
                    nc.scalar.activation(out=vrM, in_=vr,
                                         func=mybir.ActivationFunctionType.Copy,
                                         bias=float(MAGIC))
                    vr16 = pool.tile([P, CH], F16, tag="vr16")
                    nc.scalar.activation(out=vr16, in_=vrM,
                                         func=mybir.ActivationFunctionType.Copy,
                                         bias=float(-MAGIC))
                    # clamp to sentinels FIRST, then add penalties (PEN >
                    # sentinel span) -- keeps every value f16-int-exact and
                    # guarantees masked points never collide with the window.
                    nc.vector.tensor_scalar(out=vr16, in0=vr16, scalar1=SENT_HI,
                                            scalar2=SENT_LO, op0=A.min, op1=A.max)

                    # ---- masks -> penalties on vr16 ----
                    wY = pool.tile([P, CH], F32, tag="wY")
                    if need_ay:
                        nc.vector.tensor_tensor(out=wY, in0=dC,
                                                in1=ay_t[:, csl], op=A.mult)
                        nc.vector.scalar_tensor_tensor(
                            out=wY, in0=dC, scalar=by_ap, in1=wY,
                            op0=A.mult, op1=A.add)
                    else:
                        nc.scalar.activation(out=wY, in_=dC,
                                             func=mybir.ActivationFunctionType.Copy,
                                             bias=0.0, scale=by_ap)
                    vio = pool.tile([P, CH], F16, tag="vio")
                    ad = pool.tile([P, CH], F32, tag="ad")
                    nc.scalar.activation(out=ad, in_=dC,
                                         func=mybir.ActivationFunctionType.Abs)
                    for src_t, thr, cmp in ((wY, float(u_hi), A.is_ge),
                                            (wY, float(u_lo), A.is_le),
                                            (ad, float(NEAR_TH), A.is_lt),
                                            (ad, float(FAR_TH), A.is_ge)):
                        nc.vector.tensor_scalar(out=vio, in0=src_t, scalar1=thr,
                                                scalar2=PEN, op0=cmp, op1=A.mult)
                        nc.vector.tensor_tensor(out=vr16, in0=vr16, in1=vio,
                                                op=A.add)

                    # ---- one-hot + matmul accumulate ----
                    G = 32
                    n_groups = sg_per_cc // G
                    for g2 in range(n_groups):
                        sl = slice(g2 * G * nb, (g2 + 1) * G * nb)
                        lhsT = {}
                        for ri, (r0, Wr) in enumerate(r_chunks):
                            lt = ohpool.tile([P, G * nb * Wr], F16,
                                             tag=f"lh{ri}", name=f"lh{ri}")
                            nc.vector.tensor_tensor(
                                out=lt.rearrange("p (n w) -> p n w", w=Wr),
                                in0=vr16[:, sl][:, :, None].broadcast_to([P, G * nb, Wr]),
                                in1=ior[ri][:, None, :].broadcast_to([P, G * nb, Wr]),
                                op=A.is_equal)
                            lhsT[ri] = lt
                        rhs = {}
                        if sgc is None:
                            for ci, (c0, Wc) in enumerate(c_chunks):
                                rh = ohpool.tile([P, G * nb * Wc], F16,
                                                 tag=f"rh{ci}", name=f"rh{ci}")
                                nc.vector.tensor_tensor(
                                    out=rh.rearrange("p (n w) -> p n w", w=Wc),
                                    in0=vc16[:, sl][:, :, None].broadcast_to([P, G * nb, Wc]),
                                    in1=ioc[ci][:, None, :].broadcast_to([P, G * nb, Wc]),
                                    op=A.is_equal)
                                rhs[ci] = rh
                        else:
                            WCOL = sgc["Wcol"]
                            s_base = cc * sg_per_cc + g2 * G
                            rh = ohpool.tile([P, G * nb * WCOL], F16,
                                             tag="rh0", name="rh0")
                            vcv = vc16[:, sl].rearrange("p (g n) -> p g n", g=G)
                            iov = iocf[:, s_base * WCOL:(s_base + G) * WCOL] \
                                .rearrange("p (g w) -> p g w", g=G)
                            nc.vector.tensor_tensor(
                                out=rh.rearrange("p (g n w) -> p g n w", g=G, w=WCOL),
                                in0=vcv[:, :, :, None].broadcast_to([P, G, nb, WCOL]),
                                in1=iov[:, :, None, :].broadcast_to([P, G, nb, WCOL]),
                                op=A.is_equal)
                            rhs[0] = rh
                        for k in range(G):
                            s = cc * sg_per_cc + g2 * G + k
                            last = (t == n_t - 1) and (s == n_super - 1)
                            for ci, (c0, Wc) in enumerate(c_chunks):
                                for ri, (r0, Wr) in enumerate(r_chunks):
                                    if sgc is None:
                                        nc.tensor.matmul(
                                            psum[(ri, ci)],
                                            lhsT[ri][:, k * nb * Wr:(k + 1) * nb * Wr],
                                            rhs[ci][:, k * nb * Wc:(k + 1) * nb * Wc],
                                            start=(s == 0 and t == 0),
                                            stop=last)
                                    else:
                                        WCOL = sgc["Wcol"]
                                        o_s = sgc["bases"][s] - c0
                                        out_ap = psum[(ri, ci)].rearrange(
                                            "m (n q) -> m n q", q=Wc)[:, :, o_s:o_s + WCOL]
                                        nc.tensor.matmul(
                                            out_ap,
                                            lhsT[ri][:, k * nb * Wr:(k + 1) * nb * Wr],
                                            rhs[ci][:, k * nb * WCOL:(k + 1) * nb * WCOL],
                                            start=False,
                                            stop=last)
            # ---- extract: cross-block fold ----
            for ri, (r0, Wr) in enumerate(r_chunks):
                for ci, (c0, Wc) in enumerate(c_chunks):
                    psb = pool.tile([nb * Wr, nb * Wc], F32, tag="psb")
                    nc.vector.tensor_tensor(
                        out=psb.rearrange("p (n w) -> p n w", n=nb),
                        in0=psum[(ri, ci)].rearrange("p (n w) -> p n w", n=nb),
                        in1=m8[ri][:, :, None].broadcast_to([nb * Wr, nb, Wc]),
                        op=A.mult)
                    ps2 = psum2_pool.tile([Wr, nb * Wc], F32, tag="ps2")
                    nc.tensor.matmul(ps2, sel[ri], psb, start=True, stop=True)
                    o2 = pool.tile([Wr, nb * Wc], F32, tag="o2")
                    nc.vector.tensor_copy(out=o2, in_=ps2)
                    acc = pool.tile([Wr, Wc], F32, tag="acc")
                    nc.vector.tensor_copy(out=acc, in_=o2[:, 0:Wc])
                    for b in range(1, nb):
                        nc.vector.tensor_tensor(out=acc, in0=acc,
                                                in1=o2[:, b * Wc:(b + 1) * Wc],
                                                op=A.add)
                    nc.sync.dma_start(out=win_dram[(ri, ci)], in_=acc)

    nc.compile()
    nc.m = get_hw_module(nc.m)
    _phase1_cache[key] = nc
    return nc


# =====================================================================
# host fallback (exact reference replication, used for gate corner cases)
# =====================================================================
def _host_reference(depth, pose):
    d = np.asarray(depth, _dt)
    pose = np.asarray(pose, _dt)
    sx = _sxv()
    sy = _syv()
    px = d * sx[None, :]
    py = d * sy[:, None]
    pz = d
    mask1 = (np.abs(pz) < FAR_TH) & (np.abs(pz) >= NEAR_TH)
    ones = np.ones_like(d)
    gx = pose[0, 0] * px + pose[0, 1] * py + pose[0, 2] * pz + pose[0, 3] * ones
    gy = pose[1, 0] * px + pose[1, 1] * py + pose[1, 2] * pz + pose[1, 3] * ones
    gz = pose[2, 0] * px + pose[2, 1] * py + pose[2, 2] * pz + pose[2, 3] * ones
    gy = -gy + CAMERA_HEIGHT
    mask2 = mask1 & (gy > H_MIN) & (gy < H_MAX)
    r = np.round(gz / _dt(0.1) + _dt(SHIFT)).astype(np.int64)
    c = np.round(gx / _dt(0.1) + _dt(SHIFT)).astype(np.int64)
    inb = (r >= 0) & (r < M) & (c >= 0) & (c < M)
    valid = mask2 & inb
    flat = np.where(valid, r * M + c, 0)
    hist = np.bincount(flat.ravel(), weights=valid.ravel().astype(np.float64),
                       minlength=M * M).astype(_dt).reshape(M, M)
    n1 = int(mask1.sum())
    n2 = int(mask2.sum())
    ok = (n1 >= 20) and (n2 > MIN_PTS)
    return hist if ok else np.zeros((M, M), _dt)


# =====================================================================
# main entry
# =====================================================================
_static_cache = {}


def _make_cfg(plan, dlo, dhi):
    r_lo, r_hi = plan["rbox"]
    c_lo, c_hi = plan["cbox"]
    boxw_r = r_hi - r_lo + 1
    boxw_c = c_hi - c_lo + 1

    Wr_u = min(128, _pad_to(boxw_r, 2))
    nb = 1
    while nb < 8 and 2 * nb * Wr_u <= P:
        nb *= 2
    r_chunks = _chunks(r_lo, r_hi, Wr_u)
    r_chunks = [(r0, Wr_u) for (r0, w) in r_chunks]
    c_cap = (512 // nb) & ~1
    c_chunks = _chunks(c_lo, c_hi, c_cap)
    c_chunks = [(c0, _pad_to(w, 2)) for (c0, w) in c_chunks]
    assert len(r_chunks) * len(c_chunks) <= 6, "window too large for PSUM"

    sgc = None
    if len(c_chunks) == 1:
        n_super_all = W // nb
        ax_v, bx_v = plan["ax"], plan["bx"]
        kx_v = plan["kx"]
        bxa = np.concatenate([bx_v[t * P:(t + 1) * P] for t in plan["active"]]) \
            if plan["active"] else bx_v
        bx_int = (float(bxa.min()), float(bxa.max()))
        d_int = _valid_d(dlo, dhi)
        bases = []
        tops = []
        for s in range(n_super_all):
            ag = ax_v[s * nb:(s + 1) * nb]
            ci_ = _iadd((float(ag.min()), float(ag.max())), bx_int)
            g = _iadd(_imul(d_int, ci_), (kx_v, kx_v))
            v = (10.0 * g[0] + SHIFT, 10.0 * g[1] + SHIFT)
            bases.append(max(int(np.floor(v[0])) - 1, c_lo))
            tops.append(min(int(np.ceil(v[1])) + 1, c_lo + c_chunks[0][1] - 1))
        Wcol = _pad_to(max(t - b + 1 for b, t in zip(bases, tops)), 2)
        bases = [min(b, c_lo + c_chunks[0][1] - Wcol) for b in bases]
        if Wcol + 4 < c_chunks[0][1]:
            sgc = dict(Wcol=Wcol, bases=tuple(bases))

    active = plan["active"]
    n_t = (len(active) + N_CORES - 1) // N_CORES

    ax, bx = plan["ax"], plan["bx"]
    ay, by = plan["ay"], plan["by"]
    az, bz = plan["az"], plan["bz"]
    ax_const = float(ax[0]) if np.all(ax == ax[0]) else None
    az_const = float(az[0]) if np.all(az == az[0]) else None
    bx_zero = bool(np.all(bx == 0))
    bz_zero = bool(np.all(bz == 0))
    ay_zero = bool(np.all(ay == 0))

    cfg = dict(
        key=(n_t, nb, tuple(r_chunks), tuple(c_chunks),
             ax_const, az_const, bx_zero, bz_zero, ay_zero,
             plan["kx"], plan["kz"], plan["u_lo"], plan["u_hi"],
             (sgc["Wcol"], sgc["bases"]) if sgc else None),
        n_t=n_t, nb=nb, r_chunks=r_chunks, c_chunks=c_chunks,
        ax_const=ax_const, az_const=az_const,
        bx_zero=bx_zero, bz_zero=bz_zero, ay_zero=ay_zero,
        kx=plan["kx"], kz=plan["kz"], u_lo=plan["u_lo"], u_hi=plan["u_hi"],
        sgc=sgc)
    return cfg


def kernel(depth, pose):
    t_start = _time.perf_counter()
    depth = np.asarray(depth, _dt)
    pose = np.asarray(pose, _dt)
    assert depth.shape == (H, W)

    # ---- host planning: depth range + quantization grid ----
    dmin = float(depth.min())
    dmax = float(depth.max())
    # quantization domain, snapped to a coarse grid for config stability
    q_lo = max(math.floor(dmin * 16.0) / 16.0, -CLAMP)
    q_hi = min(math.ceil(dmax * 16.0) / 16.0, CLAMP)
    if q_hi <= q_lo:
        q_hi = q_lo + 1.0 / 16.0
    q_scale = (q_hi - q_lo) / QLEV
    # plan over the clamped range, padded by one quantization step
    dlo = max(dmin, -float(FAR_TH)) - q_scale
    dhi = min(dmax, float(FAR_TH)) + q_scale

    plan = _plan(pose, dlo, dhi)
    if plan is None or not plan["active"]:
        return _host_reference(depth, pose)

    cfg = _make_cfg(plan, dlo, dhi)

    # Padded slab rows (when active tiles don't divide evenly) carry q=0,
    # which dequantizes to d=q_lo with by=0. Verify such rows are always
    # masked (near/far or height band); else fall back to the exact host path.
    n_fill = cfg["n_t"] * N_CORES - len(plan["active"])
    if n_fill > 0:
        d_f = q_lo
        safe = abs(d_f) < float(NEAR_TH) or abs(d_f) >= float(FAR_TH)
        if not safe:
            ay_v = plan["ay"]
            w_lo = min(d_f * float(ay_v.min()), d_f * float(ay_v.max()))
            w_hi = max(d_f * float(ay_v.min()), d_f * float(ay_v.max()))
            safe = (w_hi <= plan["u_lo"]) or (w_lo >= plan["u_hi"])
        if not safe:
            return _host_reference(depth, pose)

    nc = _build_phase1(cfg)

    r_chunks = cfg["r_chunks"]
    c_chunks = cfg["c_chunks"]
    nb = cfg["nb"]
    n_t = cfg["n_t"]
    sgc = cfg["sgc"]
    active = plan["active"]
    ax, bx = plan["ax"], plan["bx"]
    ay, by = plan["ay"], plan["by"]
    az, bz = plan["az"], plan["bz"]
    ax_const = cfg["ax_const"]
    az_const = cfg["az_const"]
    ay_zero = cfg["ay_zero"]

    # ---- shared aux inputs (static per cfg+pose+quant grid: cached) ----
    static_key = (cfg["key"], pose.tobytes(), q_lo, q_scale)
    cached = _static_cache.get("k") == static_key
    if not cached:
        aux_inputs = {}
        for ri, (r0, Wr) in enumerate(r_chunks):
            aux_inputs[f"ior{ri}"] = \
                (r0 + np.arange(Wr)).astype(np.float16)[None, :]
            pidx = np.arange(nb * Wr)
            s = np.zeros((nb * Wr, Wr), _dt)
            s[pidx, pidx % Wr] = 1.0
            aux_inputs[f"sel{ri}"] = s
            mm = np.zeros((nb * Wr, nb), _dt)
            mm[pidx, pidx // Wr] = 1.0
            aux_inputs[f"m8_{ri}"] = mm
        if sgc is None:
            for ci, (c0, Wc) in enumerate(c_chunks):
                aux_inputs[f"ioc{ci}"] = \
                    (c0 + np.arange(Wc)).astype(np.float16)[None, :]
        else:
            Wcol = sgc["Wcol"]
            vals = (np.asarray(sgc["bases"], np.float32)[:, None]
                    + np.arange(Wcol, dtype=np.float32)[None, :]).astype(np.float16)
            aux_inputs["iocf"] = vals.reshape(1, -1)
        if ax_const is None:
            aux_inputs["axr"] = ax[None, :]
        if az_const is None:
            aux_inputs["azr"] = az[None, :]
        if not ay_zero:
            aux_inputs["ayr"] = ay[None, :]
        bcols_percore = []
        for g in range(N_CORES):
            tiles = active[g::N_CORES]
            bcols = np.zeros((P, 4 * n_t + 2), _dt)
            for k, t in enumerate(tiles):
                bcols[:, 4 * k + 0] = bx[t * P:(t + 1) * P]
                bcols[:, 4 * k + 1] = by[t * P:(t + 1) * P]
                bcols[:, 4 * k + 2] = bz[t * P:(t + 1) * P]
            bcols[:, 4 * n_t + 0] = q_scale
            bcols[:, 4 * n_t + 1] = q_lo
            bcols_percore.append(bcols)
        _static_cache.clear()
        _static_cache.update(k=static_key, aux=aux_inputs, bcols=bcols_percore)
    aux_inputs = _static_cache["aux"]
    bcols_percore = _static_cache["bcols"]

    # ---- per-core inputs: quantized depth slabs ----
    inv_scale = _dt(1.0 / q_scale)
    # round-half-up via +0.5 and truncation; values are >= 0 after -q_lo
    q_off = _dt(0.5 - q_lo / q_scale)
    need_clip = dmin < -CLAMP or dmax > CLAMP
    contig = active == list(range(active[0], active[0] + len(active))) \
        and len(active) == n_t * N_CORES
    if contig:
        # single vectorized quantization over the contiguous active block
        rows = depth[active[0] * P:(active[0] + len(active)) * P, :]
        if need_clip:
            rows = np.clip(rows, -CLAMP, CLAMP)
        qall = (rows * inv_scale + q_off).astype(np.uint16)
    in_maps = []
    for g in range(N_CORES):
        tiles = active[g::N_CORES]
        if contig:
            # tiles are active[0]+g, active[0]+g+8, ... -> strided view rows
            dslab = np.concatenate(
                [qall[(t - active[0]) * P:(t - active[0] + 1) * P, :]
                 for t in tiles], axis=0) if n_t > 1 else \
                qall[(tiles[0] - active[0]) * P:(tiles[0] - active[0] + 1) * P, :]
        else:
            dslab = np.zeros((n_t * P, W), np.uint16)
            for k, t in enumerate(tiles):
                rows = depth[t * P:(t + 1) * P, :]
                if need_clip:
                    rows = np.clip(rows, -CLAMP, CLAMP)
                dslab[k * P:(k + 1) * P, :] = \
                    (rows * inv_scale + q_off).astype(np.uint16)
        im = {"d1": dslab, "bcols": bcols_percore[g]}
        im.update(aux_inputs)
        in_maps.append(im)

    LAST_EXEC_NS["prep_wall"] = int((_time.perf_counter() - t_start) * 1e9)
    _t0 = _time.perf_counter()
    res = run_bass_kernel_spmd(nc, in_maps, core_ids=list(range(N_CORES)),
                               trace=TRACE)
    LAST_EXEC_NS["phase1_wall"] = int((_time.perf_counter() - _t0) * 1e9)
    if TRACE:
        LAST_EXEC_NS["phase1"] = res.exec_time_ns

    hist = np.zeros((M, M), _dt)
    for ri, (r0, Wr) in enumerate(r_chunks):
        for ci, (c0, Wc) in enumerate(c_chunks):
            tot = np.zeros((Wr, Wc), np.float64)
            for r in res.results:
                tot += r[f"win{ri}_{ci}"]
            rs = max(r0, 0)
            re = min(r0 + Wr, M)
            cs = max(c0, 0)
            ce = min(c0 + Wc, M)
            if rs < re and cs < ce:
                hist[rs:re, cs:ce] = tot[rs - r0:re - r0, cs - c0:ce - c0]

    if hist.sum() < 4096:
        return _host_reference(depth, pose)
    return hist.astype(_dt)


if __name__ == "__main__":
    rng = np.random.default_rng(0)
    d = rng.random((H, W), _dt)
    p = np.eye(4, dtype=_dt)
    out = kernel(d, p)
    print("sum", out.sum(), "nonzero", (out > 0).sum())


# revision 10
# speedup vs baseline: 1.0699x; 1.0699x over previous
"""Trainium2 Bass kernel for nn_DirectDepthMapper (histogram_binning).

Pipeline (matches reference.py):
  depth (H,W) -> per-pixel point (px,py,pz) -> pose transform -> masks ->
  (r,c) = round(g{z,x}/0.1 + 200) -> 400x400 histogram of valid points.

Strategy:
  - Scatter-add reformulated as windowed one-hot construction (DVE
    is_equal against iota rows, masked points pushed out of the window by
    arithmetic penalties) contracted on the TensorEngine:
    hist_win = sum_blocks ohR^T @ ohC accumulated in PSUM.
  - The active window (bounding box of reachable bins) is derived on the
    host from a clamped min/max of depth (cheap numpy) + interval
    arithmetic over the pose coefficients; row-tiles that cannot pass the
    height-band mask are skipped and the rest are balanced over 8 cores.
  - Depth ships as affine-quantized u16 (dequantized on device); all
    replicated constants (per-column coefficient rows, per-supergroup
    column iotas) ship as single rows and are partition-broadcast on
    device, so per-core transfer is ~0.5MB.
  - One SPMD call; each core emits its partial window histogram; the host
    sums 8 small windows into the 400x400 output.

Self-contained: hardcodes H=W=2048, 8 cores.
"""
import hashlib
import math
import os
import time as _time

import numpy as np

import jax

# Persistent compilation cache: lets warm calls (and fresh processes on the
# same machine) skip the client-side BIR->NEFF recompile entirely.
try:
    jax.config.update("jax_compilation_cache_dir",
                      os.environ.get("BASS_JAX_CACHE_DIR", "/tmp/bass_jax_cache"))
    jax.config.update("jax_persistent_cache_min_compile_time_secs", 0.0)
    jax.config.update("jax_persistent_cache_min_entry_size_bytes", 0)
except Exception:
    pass

import concourse.bass as bass
import concourse.bacc as bacc
import concourse.mybir as mybir
import concourse.tile as tile
from concourse.bass_interp import get_hw_module
from concourse.bass_utils import run_bass_kernel_spmd

# In-process memo of the HLO->NEFF compile hook (pure function of the HLO
# bytes) as insurance for when the persistent cache misses.
try:
    import libneuronxla
    from concourse import bass2jax as _b2j
    _b2j.install_neuronx_cc_hook()
    if not getattr(libneuronxla, "_bass_memo_cc", None):
        _inner_cc = libneuronxla.neuronx_cc
        _cc_memo = {}

        def _memo_cc(code, code_format, platform_version, file_prefix):
            key = (hashlib.sha256(code).digest(), bytes(code_format),
                   str(platform_version))
            if key not in _cc_memo:
                _cc_memo[key] = _inner_cc(code, code_format, platform_version,
                                          file_prefix)
            return _cc_memo[key]

        libneuronxla.neuronx_cc = _memo_cc
        libneuronxla._bass_memo_cc = True
        _b2j.install_neuronx_cc_hook = lambda: None
except Exception:
    pass

# ---------------- problem constants (from reference.py) ----------------
H = W = 2048
N_CORES = 8
NEAR_TH = np.float32(0.1)
FAR_TH = np.float32(4.0)
H_MIN = np.float32(0.0)
H_MAX = np.float32(1.0)
CAMERA_HEIGHT = np.float32(0.0)
CELLS = int(math.ceil(40.0 / 0.1)) + 1   # 401
M = CELLS - 1                            # 400
SHIFT = math.floor(CELLS / 2.0)          # 200
MIN_PTS = 10

FX = np.float32(W / 2.0)
FY = np.float32(H / 2.0)
CX = int(FX) - 1
CY = int(FY) - 1

MAGIC = np.float32(1.5 * 2**23)          # fp32 round-to-nearest-int trick
CLAMP = 4.25                             # quantization clamp (> FAR_TH)
QLEV = 65535.0

# set by test harness for profiling; kernel() stores wall times here
TRACE = False
LAST_EXEC_NS = {}
P = 128                                  # partitions
ROW_TILES = H // P                       # 16
F32 = mybir.dt.float32
F16 = mybir.dt.float16
U16 = mybir.dt.uint16

_dt = np.float32


def _sxv():
    return ((np.arange(W, dtype=np.float64) - CX) / np.float64(FX)).astype(_dt)


def _syv():
    return ((np.arange(H, dtype=np.float64) - CY) / np.float64(FY)).astype(_dt)


# =====================================================================
# host-side interval arithmetic (plan the bin window + active tiles)
# =====================================================================
def _imul(a, b):
    c = [a[0] * b[0], a[0] * b[1], a[1] * b[0], a[1] * b[1]]
    return (min(c), max(c))


def _iadd(a, b):
    return (a[0] + b[0], a[1] + b[1])


def _coef_rows(pose, row):
    """a_i = pose[row,0]*sxv_i + pose[row,2]; b_j = pose[row,1]*syv_j"""
    p = np.asarray(pose, _dt)
    a = (p[row, 0] * _sxv() + p[row, 2]).astype(_dt)
    b = (p[row, 1] * _syv()).astype(_dt)
    k = float(p[row, 3])
    return a, b, k


def _valid_d(dlo, dhi):
    """hull of [dlo,dhi] restricted to the mask1-valid set |d| in [0.1, 4]."""
    lo, hi = None, None
    for a, b in ((-float(FAR_TH), -float(NEAR_TH)), (float(NEAR_TH), float(FAR_TH))):
        s, e = max(a, dlo), min(b, dhi)
        if s <= e:
            lo = s if lo is None else min(lo, s)
            hi = e if hi is None else max(hi, e)
    if lo is None:
        return None
    return (lo, hi)


def _plan(pose, dlo, dhi):
    d_int = _valid_d(dlo, dhi)
    if d_int is None:
        return None
    ax, bx, kx = _coef_rows(pose, 0)   # gx
    ay, by, ky = _coef_rows(pose, 1)   # gy raw
    az, bz, kz = _coef_rows(pose, 2)   # gz

    def box_for(a, b, k):
        c_int = _iadd((float(a.min()), float(a.max())),
                      (float(b.min()), float(b.max())))
        g = _iadd(_imul(d_int, c_int), (k, k))
        v = (10.0 * g[0] + SHIFT, 10.0 * g[1] + SHIFT)
        lo = int(np.floor(v[0])) - 1
        hi = int(np.ceil(v[1])) + 1
        return max(lo, -1), min(hi, M)

    rbox = box_for(az, bz, kz)
    cbox = box_for(ax, bx, kx)
    if rbox[0] > rbox[1] or cbox[0] > cbox[1]:
        return None

    u_hi = float(CAMERA_HEIGHT - ky - H_MIN)   # valid iff u_lo < w < u_hi
    u_lo = float(CAMERA_HEIGHT - ky - H_MAX)
    a_int = (float(ay.min()), float(ay.max()))
    active = []
    for t in range(ROW_TILES):
        bt = by[t * P:(t + 1) * P]
        c_int = _iadd(a_int, (float(bt.min()), float(bt.max())))
        w_int = _imul(d_int, c_int)
        if w_int[0] < u_hi and w_int[1] > u_lo:
            active.append(t)
    return dict(rbox=rbox, cbox=cbox, active=active,
                ax=ax, bx=bx, kx=kx, ay=ay, by=by, ky=ky,
                az=az, bz=bz, kz=kz, u_lo=u_lo, u_hi=u_hi)


def _pad_to(x, m):
    return ((x + m - 1) // m) * m


def _chunks(lo, hi, cap):
    out = []
    x = lo
    while x <= hi:
        wdt = min(cap, hi - x + 1)
        out.append((x, wdt))
        x += wdt
    return out


# =====================================================================
# phase 1 kernel builder
# =====================================================================
_phase1_cache = {}


def _layouts(cfg):
    """Segment layouts of the merged replicated-row inputs."""
    l16 = []
    for ri, (r0, Wr) in enumerate(cfg["r_chunks"]):
        l16.append((f"ior{ri}", Wr))
    if cfg["sgc"] is None:
        for ci, (c0, Wc) in enumerate(cfg["c_chunks"]):
            l16.append((f"ioc{ci}", Wc))
    else:
        l16.append(("iocf", (W // cfg["nb"]) * cfg["sgc"]["Wcol"]))
    l32 = []
    if cfg["ax_const"] is None:
        l32.append(("axr", W))
    if cfg["az_const"] is None:
        l32.append(("azr", W))
    if not cfg["ay_zero"]:
        l32.append(("ayr", W))
    return l16, l32


def _build_phase1(cfg):
    key = cfg["key"]
    if key in _phase1_cache:
        return _phase1_cache[key]

    n_t = cfg["n_t"]
    nb = cfg["nb"]
    r_chunks = cfg["r_chunks"]      # list of (r0, Wr)
    c_chunks = cfg["c_chunks"]      # list of (c0, Wc)
    ax_const = cfg["ax_const"]      # float or None
    az_const = cfg["az_const"]
    bx_zero = cfg["bx_zero"]
    bz_zero = cfg["bz_zero"]
    ay_zero = cfg["ay_zero"]
    kx = cfg["kx"]
    kz = cfg["kz"]
    u_lo = cfg["u_lo"]
    u_hi = cfg["u_hi"]
    sgc = cfg.get("sgc")          # per-supergroup c windows: (Wcol, bases)

    nc = bacc.Bacc("TRN2", target_bir_lowering=False, debug=False,
                   num_devices=N_CORES)
    d_dram = nc.dram_tensor("d1", [n_t * P, W], U16, kind="ExternalInput").ap()
    # per-row (partition) coefficient columns, packed [P, 4*n_t + 2]
    # (bx, by, bz, 0) per tile, then (qscale, qlo)
    b_dram = nc.dram_tensor("bcols", [P, 4 * n_t + 2], F32,
                            kind="ExternalInput").ap()
    need_ax = ax_const is None
    need_az = az_const is None
    need_ay = not ay_zero
    # merged row tensors: one f16 row (r/c iotas), one f32 row (coef rows)
    l16, l32 = _layouts(cfg)
    L16 = sum(n for _, n in l16)
    L32 = sum(n for _, n in l32)
    rowf16_dram = nc.dram_tensor("rowf16", [1, L16], F16,
                                 kind="ExternalInput").ap()
    if L32:
        rowf32_dram = nc.dram_tensor("rowf32", [1, L32], F32,
                                     kind="ExternalInput").ap()
    # merged per-partition constants: [sel_ri | m8_ri] per r chunk
    Wr0 = r_chunks[0][1]
    pm_dram = nc.dram_tensor("pm", [nb * Wr0, len(r_chunks) * (Wr0 + nb)], F32,
                             kind="ExternalInput").ap()
    win_dram = {}
    for ri, (r0, Wr) in enumerate(r_chunks):
        for ci, (c0, Wc) in enumerate(c_chunks):
            win_dram[(ri, ci)] = nc.dram_tensor(
                f"win{ri}_{ci}", [Wr, Wc], F32, kind="ExternalOutput").ap()

    A = mybir.AluOpType
    SENT_LO = float(min(r0 for r0, _ in r_chunks) - 5)
    SENT_HI = float(max(r0 + w for r0, w in r_chunks) + 4)
    PEN = 256.0  # > sentinel span (Wr+9 <= 137), 4*PEN + |SENT| < 2048 (f16 int-exact)

    with tile.TileContext(nc) as tc:
        with tc.tile_pool(name="const", bufs=1) as cpool, \
             tc.tile_pool(name="sbuf", bufs=2) as pool, \
             tc.tile_pool(name="oh", bufs=2) as ohpool, \
             tc.tile_pool(name="psum", bufs=1, space="PSUM") as psum_pool, \
             tc.tile_pool(name="psum2", bufs=2, space="PSUM") as psum2_pool:

            # ---- constants (two broadcasts cover every replicated row) ----
            r16a = cpool.tile([1, L16], F16, tag="r16a")
            nc.sync.dma_start(out=r16a, in_=rowf16_dram)
            r16 = cpool.tile([P, L16], F16, tag="r16")
            nc.gpsimd.partition_broadcast(r16, r16a)
            if L32:
                r32a = cpool.tile([1, L32], F32, tag="r32a")
                nc.sync.dma_start(out=r32a, in_=rowf32_dram)
                r32 = cpool.tile([P, L32], F32, tag="r32")
                nc.gpsimd.partition_broadcast(r32, r32a)
            seg16 = {}
            off = 0
            for name, ln in l16:
                seg16[name] = r16[:, off:off + ln]
                off += ln
            seg32 = {}
            off = 0
            for name, ln in l32:
                seg32[name] = r32[:, off:off + ln]
                off += ln
            ior = {ri: seg16[f"ior{ri}"] for ri, _ in enumerate(r_chunks)}
            if sgc is None:
                ioc = {ci: seg16[f"ioc{ci}"] for ci, _ in enumerate(c_chunks)}
            else:
                iocf = seg16["iocf"]
                zlh = cpool.tile([P, nb * r_chunks[0][1]], F16, tag="zlh")
                nc.vector.memset(zlh, 0.0)
                zrh = cpool.tile([P, nb * c_chunks[0][1]], F16, tag="zrh")
                nc.vector.memset(zrh, 0.0)
            if need_ax:
                ax_t = seg32["axr"]
            if need_az:
                az_t = seg32["azr"]
            if need_ay:
                ay_t = seg32["ayr"]
            pm = cpool.tile([nb * Wr0, len(r_chunks) * (Wr0 + nb)], F32,
                            tag="pm")
            nc.sync.dma_start(out=pm, in_=pm_dram)
            sel = {}
            m8 = {}
            for ri, (r0, Wr) in enumerate(r_chunks):
                base = ri * (Wr0 + nb)
                sel[ri] = pm[:, base:base + Wr]
                m8[ri] = pm[:, base + Wr:base + Wr + nb]
            bcols = cpool.tile([P, 4 * n_t + 2], F32, tag="bcols")
            nc.sync.dma_start(out=bcols, in_=b_dram)
            qs_ap = bcols[:, 4 * n_t + 0:4 * n_t + 1]
            ql_ap = bcols[:, 4 * n_t + 1:4 * n_t + 2]

            psum = {}
            for ri, (r0, Wr) in enumerate(r_chunks):
                for ci, (c0, Wc) in enumerate(c_chunks):
                    psum[(ri, ci)] = psum_pool.tile([nb * Wr, nb * Wc], F32,
                                                    tag=f"ps{ri}_{ci}",
                                                    name=f"ps{ri}_{ci}")

            n_super = W // nb
            if sgc is not None:
                for ri, (r0, Wr) in enumerate(r_chunks):
                    nc.tensor.matmul(psum[(ri, 0)], zlh, zrh,
                                     start=True, stop=False)
            CH = 1024                     # column chunk for pipelining
            n_cc = W // CH
            sg_per_cc = CH // nb
            for t in range(n_t):
                q = pool.tile([P, W], U16, tag="q")
                nc.sync.dma_start(out=q, in_=d_dram[t * P:(t + 1) * P, :])
                # dequant: d = qscale*q + qlo
                d = pool.tile([P, W], F32, tag="d")
                nc.scalar.activation(out=d, in_=q,
                                     func=mybir.ActivationFunctionType.Copy,
                                     bias=0.0, scale=qs_ap)
                nc.vector.tensor_scalar(out=d, in0=d, scalar1=ql_ap,
                                        scalar2=None, op0=A.add)
                bx_ap = bcols[:, 4 * t + 0:4 * t + 1]
                by_ap = bcols[:, 4 * t + 1:4 * t + 2]
                bz_ap = bcols[:, 4 * t + 2:4 * t + 3]

                for cc in range(n_cc):
                    csl = slice(cc * CH, (cc + 1) * CH)
                    dC = d[:, csl]

                    # ---- c index ----
                    vc = pool.tile([P, CH], F32, tag="vc")
                    if ax_const is None:
                        tC = pool.tile([P, CH], F32, tag="tC")
                        nc.vector.tensor_tensor(out=tC, in0=dC,
                                                in1=ax_t[:, csl], op=A.mult)
                        if not bx_zero:
                            nc.vector.scalar_tensor_tensor(
                                out=tC, in0=dC, scalar=bx_ap, in1=tC,
                                op0=A.mult, op1=A.add)
                        nc.vector.tensor_scalar(
                            out=vc, in0=tC, scalar1=10.0,
                            scalar2=float(SHIFT + 10.0 * kx),
                            op0=A.mult, op1=A.add)
                    else:
                        if not bx_zero:
                            tC = pool.tile([P, CH], F32, tag="tC")
                            nc.vector.tensor_scalar(out=tC, in0=dC, scalar1=bx_ap,
                                                    scalar2=None, op0=A.mult)
                            nc.vector.scalar_tensor_tensor(
                                out=tC, in0=dC, scalar=float(ax_const), in1=tC,
                                op0=A.mult, op1=A.add)
                            nc.vector.tensor_scalar(
                                out=vc, in0=tC, scalar1=10.0,
                                scalar2=float(SHIFT + 10.0 * kx),
                                op0=A.mult, op1=A.add)
                        else:
                            nc.vector.tensor_scalar(
                                out=vc, in0=dC, scalar1=float(10.0 * ax_const),
                                scalar2=float(SHIFT + 10.0 * kx),
                                op0=A.mult, op1=A.add)
                    vcM = pool.tile([P, CH], F32, tag="vcM")
                    nc.scalar.activation(out=vcM, in_=vc,
                                         func=mybir.ActivationFunctionType.Copy,
                                         bias=float(MAGIC))
                    vc16 = pool.tile([P, CH], F16, tag="vc16")
                    nc.scalar.activation(out=vc16, in_=vcM,
                                         func=mybir.ActivationFunctionType.Copy,
                                         bias=float(-MAGIC))

                    # ---- r index ----
                    vr = pool.tile([P, CH], F32, tag="vr")
                    if az_const is None:
                        tZ = pool.tile([P, CH], F32, tag="tZ")
                        nc.vector.tensor_tensor(out=tZ, in0=dC,
                                                in1=az_t[:, csl], op=A.mult)
                        if not bz_zero:
                            nc.vector.scalar_tensor_tensor(
                                out=tZ, in0=dC, scalar=bz_ap, in1=tZ,
                                op0=A.mult, op1=A.add)
                        nc.vector.tensor_scalar(
                            out=vr, in0=tZ, scalar1=10.0,
                            scalar2=float(SHIFT + 10.0 * kz),
                            op0=A.mult, op1=A.add)
                    else:
                        if not bz_zero:
                            tZ = pool.tile([P, CH], F32, tag="tZ")
                            nc.vector.tensor_scalar(out=tZ, in0=dC, scalar1=bz_ap,
                                                    scalar2=None, op0=A.mult)
                            nc.vector.scalar_tensor_tensor(
                                out=tZ, in0=dC, scalar=float(az_const), in1=tZ,
                                op0=A.mult, op1=A.add)
                            nc.vector.tensor_scalar(
                                out=vr, in0=tZ, scalar1=10.0,
                                scalar2=float(SHIFT + 10.0 * kz),
                                op0=A.mult, op1=A.add)
                        else:
                            nc.vector.tensor_scalar(
                                out=vr, in0=dC, scalar1=float(10.0 * az_const),
                                scalar2=float(SHIFT + 10.0 * kz),
                                op0=A.mult, op1=A.add)
                    vrM = pool.tile([P, CH], F32, tag="vrM")
                    nc.scalar.activation(out=vrM, in_=vr,
                                         func=mybir.ActivationFunctionType.Copy,
                                         bias=float(MAGIC))
                    vr16 = pool.tile([P, CH], F16, tag="vr16")
                    nc.scalar.activation(out=vr16, in_=vrM,
                                         func=mybir.ActivationFunctionType.Copy,
                                         bias=float(-MAGIC))
                    # clamp to sentinels FIRST, then add penalties (PEN >
                    # sentinel span) -- keeps every value f16-int-exact and
                    # guarantees masked points never collide with the window.
                    nc.vector.tensor_scalar(out=vr16, in0=vr16, scalar1=SENT_HI,
                                            scalar2=SENT_LO, op0=A.min, op1=A.max)

                    # ---- masks -> penalties on vr16 ----
                    wY = pool.tile([P, CH], F32, tag="wY")
                    if need_ay:
                        nc.vector.tensor_tensor(out=wY, in0=dC,
                                                in1=ay_t[:, csl], op=A.mult)
                        nc.vector.scalar_tensor_tensor(
                            out=wY, in0=dC, scalar=by_ap, in1=wY,
                            op0=A.mult, op1=A.add)
                    else:
                        nc.scalar.activation(out=wY, in_=dC,
                                             func=mybir.ActivationFunctionType.Copy,
                                             bias=0.0, scale=by_ap)
                    vio = pool.tile([P, CH], F16, tag="vio")
                    ad = pool.tile([P, CH], F32, tag="ad")
                    nc.scalar.activation(out=ad, in_=dC,
                                         func=mybir.ActivationFunctionType.Abs)
                    for src_t, thr, cmp in ((wY, float(u_hi), A.is_ge),
                                            (wY, float(u_lo), A.is_le),
                                            (ad, float(NEAR_TH), A.is_lt),
                                            (ad, float(FAR_TH), A.is_ge)):
                        nc.vector.tensor_scalar(out=vio, in0=src_t, scalar1=thr,
                                                scalar2=PEN, op0=cmp, op1=A.mult)
                        nc.vector.tensor_tensor(out=vr16, in0=vr16, in1=vio,
                                                op=A.add)

                    # ---- one-hot + matmul accumulate ----
                    G = 32
                    n_groups = sg_per_cc // G
                    for g2 in range(n_groups):
                        sl = slice(g2 * G * nb, (g2 + 1) * G * nb)
                        lhsT = {}
                        for ri, (r0, Wr) in enumerate(r_chunks):
                            lt = ohpool.tile([P, G * nb * Wr], F16,
                                             tag=f"lh{ri}", name=f"lh{ri}")
                            nc.vector.tensor_tensor(
                                out=lt.rearrange("p (n w) -> p n w", w=Wr),
                                in0=vr16[:, sl][:, :, None].broadcast_to([P, G * nb, Wr]),
                                in1=ior[ri][:, None, :].broadcast_to([P, G * nb, Wr]),
                                op=A.is_equal)
                            lhsT[ri] = lt
                        rhs = {}
                        if sgc is None:
                            for ci, (c0, Wc) in enumerate(c_chunks):
                                rh = ohpool.tile([P, G * nb * Wc], F16,
                                                 tag=f"rh{ci}", name=f"rh{ci}")
                                nc.vector.tensor_tensor(
                                    out=rh.rearrange("p (n w) -> p n w", w=Wc),
                                    in0=vc16[:, sl][:, :, None].broadcast_to([P, G * nb, Wc]),
                                    in1=ioc[ci][:, None, :].broadcast_to([P, G * nb, Wc]),
                                    op=A.is_equal)
                                rhs[ci] = rh
                        else:
                            WCOL = sgc["Wcol"]
                            s_base = cc * sg_per_cc + g2 * G
                            rh = ohpool.tile([P, G * nb * WCOL], F16,
                                             tag="rh0", name="rh0")
                            vcv = vc16[:, sl].rearrange("p (g n) -> p g n", g=G)
                            iov = iocf[:, s_base * WCOL:(s_base + G) * WCOL] \
                                .rearrange("p (g w) -> p g w", g=G)
                            nc.vector.tensor_tensor(
                                out=rh.rearrange("p (g n w) -> p g n w", g=G, w=WCOL),
                                in0=vcv[:, :, :, None].broadcast_to([P, G, nb, WCOL]),
                                in1=iov[:, :, None, :].broadcast_to([P, G, nb, WCOL]),
                                op=A.is_equal)
                            rhs[0] = rh
                        for k in range(G):
                            s = cc * sg_per_cc + g2 * G + k
                            last = (t == n_t - 1) and (s == n_super - 1)
                            for ci, (c0, Wc) in enumerate(c_chunks):
                                for ri, (r0, Wr) in enumerate(r_chunks):
                                    if sgc is None:
                                        nc.tensor.matmul(
                                            psum[(ri, ci)],
                                            lhsT[ri][:, k * nb * Wr:(k + 1) * nb * Wr],
                                            rhs[ci][:, k * nb * Wc:(k + 1) * nb * Wc],
                                            start=(s == 0 and t == 0),
                                            stop=last)
                                    else:
                                        WCOL = sgc["Wcol"]
                                        o_s = sgc["bases"][s] - c0
                                        out_ap = psum[(ri, ci)].rearrange(
                                            "m (n q) -> m n q", q=Wc)[:, :, o_s:o_s + WCOL]
                                        nc.tensor.matmul(
                                            out_ap,
                                            lhsT[ri][:, k * nb * Wr:(k + 1) * nb * Wr],
                                            rhs[ci][:, k * nb * WCOL:(k + 1) * nb * WCOL],
                                            start=False,
                                            stop=last)
            # ---- extract: cross-block fold ----
            for ri, (r0, Wr) in enumerate(r_chunks):
                for ci, (c0, Wc) in enumerate(c_chunks):
                    psb = pool.tile([nb * Wr, nb * Wc], F32, tag="psb")
                    nc.vector.tensor_tensor(
                        out=psb.rearrange("p (n w) -> p n w", n=nb),
                        in0=psum[(ri, ci)].rearrange("p (n w) -> p n w", n=nb),
                        in1=m8[ri][:, :, None].broadcast_to([nb * Wr, nb, Wc]),
                        op=A.mult)
                    ps2 = psum2_pool.tile([Wr, nb * Wc], F32, tag="ps2")
                    nc.tensor.matmul(ps2, sel[ri], psb, start=True, stop=True)
                    o2 = pool.tile([Wr, nb * Wc], F32, tag="o2")
                    nc.vector.tensor_copy(out=o2, in_=ps2)
                    acc = pool.tile([Wr, Wc], F32, tag="acc")
                    nc.vector.tensor_copy(out=acc, in_=o2[:, 0:Wc])
                    for b in range(1, nb):
                        nc.vector.tensor_tensor(out=acc, in0=acc,
                                                in1=o2[:, b * Wc:(b + 1) * Wc],
                                                op=A.add)
                    nc.sync.dma_start(out=win_dram[(ri, ci)], in_=acc)

    nc.compile()
    nc.m = get_hw_module(nc.m)
    _phase1_cache[key] = nc
    return nc


# =====================================================================
# host fallback (exact reference replication, used for gate corner cases)
# =====================================================================
def _host_reference(depth, pose):
    d = np.asarray(depth, _dt)
    pose = np.asarray(pose, _dt)
    sx = _sxv()
    sy = _syv()
    px = d * sx[None, :]
    py = d * sy[:, None]
    pz = d
    mask1 = (np.abs(pz) < FAR_TH) & (np.abs(pz) >= NEAR_TH)
    ones = np.ones_like(d)
    gx = pose[0, 0] * px + pose[0, 1] * py + pose[0, 2] * pz + pose[0, 3] * ones
    gy = pose[1, 0] * px + pose[1, 1] * py + pose[1, 2] * pz + pose[1, 3] * ones
    gz = pose[2, 0] * px + pose[2, 1] * py + pose[2, 2] * pz + pose[2, 3] * ones
    gy = -gy + CAMERA_HEIGHT
    mask2 = mask1 & (gy > H_MIN) & (gy < H_MAX)
    r = np.round(gz / _dt(0.1) + _dt(SHIFT)).astype(np.int64)
    c = np.round(gx / _dt(0.1) + _dt(SHIFT)).astype(np.int64)
    inb = (r >= 0) & (r < M) & (c >= 0) & (c < M)
    valid = mask2 & inb
    flat = np.where(valid, r * M + c, 0)
    hist = np.bincount(flat.ravel(), weights=valid.ravel().astype(np.float64),
                       minlength=M * M).astype(_dt).reshape(M, M)
    n1 = int(mask1.sum())
    n2 = int(mask2.sum())
    ok = (n1 >= 20) and (n2 > MIN_PTS)
    return hist if ok else np.zeros((M, M), _dt)


# =====================================================================
# main entry
# =====================================================================
_static_cache = {}


def _make_cfg(plan, dlo, dhi):
    r_lo, r_hi = plan["rbox"]
    c_lo, c_hi = plan["cbox"]
    boxw_r = r_hi - r_lo + 1
    boxw_c = c_hi - c_lo + 1

    Wr_u = min(128, _pad_to(boxw_r, 2))
    nb = 1
    while nb < 8 and 2 * nb * Wr_u <= P:
        nb *= 2
    r_chunks = _chunks(r_lo, r_hi, Wr_u)
    r_chunks = [(r0, Wr_u) for (r0, w) in r_chunks]
    c_cap = (512 // nb) & ~1
    c_chunks = _chunks(c_lo, c_hi, c_cap)
    c_chunks = [(c0, _pad_to(w, 2)) for (c0, w) in c_chunks]
    assert len(r_chunks) * len(c_chunks) <= 6, "window too large for PSUM"

    sgc = None
    if len(c_chunks) == 1:
        n_super_all = W // nb
        ax_v, bx_v = plan["ax"], plan["bx"]
        kx_v = plan["kx"]
        bxa = np.concatenate([bx_v[t * P:(t + 1) * P] for t in plan["active"]]) \
            if plan["active"] else bx_v
        bx_int = (float(bxa.min()), float(bxa.max()))
        d_int = _valid_d(dlo, dhi)
        bases = []
        tops = []
        for s in range(n_super_all):
            ag = ax_v[s * nb:(s + 1) * nb]
            ci_ = _iadd((float(ag.min()), float(ag.max())), bx_int)
            g = _iadd(_imul(d_int, ci_), (kx_v, kx_v))
            v = (10.0 * g[0] + SHIFT, 10.0 * g[1] + SHIFT)
            bases.append(max(int(np.floor(v[0])) - 1, c_lo))
            tops.append(min(int(np.ceil(v[1])) + 1, c_lo + c_chunks[0][1] - 1))
        Wcol = _pad_to(max(t - b + 1 for b, t in zip(bases, tops)), 2)
        bases = [min(b, c_lo + c_chunks[0][1] - Wcol) for b in bases]
        if Wcol + 4 < c_chunks[0][1]:
            sgc = dict(Wcol=Wcol, bases=tuple(bases))

    active = plan["active"]
    n_t = (len(active) + N_CORES - 1) // N_CORES

    ax, bx = plan["ax"], plan["bx"]
    ay, by = plan["ay"], plan["by"]
    az, bz = plan["az"], plan["bz"]
    ax_const = float(ax[0]) if np.all(ax == ax[0]) else None
    az_const = float(az[0]) if np.all(az == az[0]) else None
    bx_zero = bool(np.all(bx == 0))
    bz_zero = bool(np.all(bz == 0))
    ay_zero = bool(np.all(ay == 0))

    cfg = dict(
        key=(n_t, nb, tuple(r_chunks), tuple(c_chunks),
             ax_const, az_const, bx_zero, bz_zero, ay_zero,
             plan["kx"], plan["kz"], plan["u_lo"], plan["u_hi"],
             (sgc["Wcol"], sgc["bases"]) if sgc else None),
        n_t=n_t, nb=nb, r_chunks=r_chunks, c_chunks=c_chunks,
        ax_const=ax_const, az_const=az_const,
        bx_zero=bx_zero, bz_zero=bz_zero, ay_zero=ay_zero,
        kx=plan["kx"], kz=plan["kz"], u_lo=plan["u_lo"], u_hi=plan["u_hi"],
        sgc=sgc)
    return cfg


def kernel(depth, pose):
    t_start = _time.perf_counter()
    depth = np.asarray(depth, _dt)
    pose = np.asarray(pose, _dt)
    assert depth.shape == (H, W)

    # ---- host planning: depth range + quantization grid ----
    dmin = float(depth.min())
    dmax = float(depth.max())
    # quantization domain, snapped to a coarse grid for config stability
    q_lo = max(math.floor(dmin * 16.0) / 16.0, -CLAMP)
    q_hi = min(math.ceil(dmax * 16.0) / 16.0, CLAMP)
    if q_hi <= q_lo:
        q_hi = q_lo + 1.0 / 16.0
    q_scale = (q_hi - q_lo) / QLEV
    # plan over the clamped range, padded by one quantization step
    dlo = max(dmin, -float(FAR_TH)) - q_scale
    dhi = min(dmax, float(FAR_TH)) + q_scale

    plan = _plan(pose, dlo, dhi)
    if plan is None or not plan["active"]:
        return _host_reference(depth, pose)

    cfg = _make_cfg(plan, dlo, dhi)

    # Padded slab rows (when active tiles don't divide evenly) carry q=0,
    # which dequantizes to d=q_lo with by=0. Verify such rows are always
    # masked (near/far or height band); else fall back to the exact host path.
    n_fill = cfg["n_t"] * N_CORES - len(plan["active"])
    if n_fill > 0:
        d_f = q_lo
        safe = abs(d_f) < float(NEAR_TH) or abs(d_f) >= float(FAR_TH)
        if not safe:
            ay_v = plan["ay"]
            w_lo = min(d_f * float(ay_v.min()), d_f * float(ay_v.max()))
            w_hi = max(d_f * float(ay_v.min()), d_f * float(ay_v.max()))
            safe = (w_hi <= plan["u_lo"]) or (w_lo >= plan["u_hi"])
        if not safe:
            return _host_reference(depth, pose)

    nc = _build_phase1(cfg)

    r_chunks = cfg["r_chunks"]
    c_chunks = cfg["c_chunks"]
    nb = cfg["nb"]
    n_t = cfg["n_t"]
    sgc = cfg["sgc"]
    active = plan["active"]
    ax, bx = plan["ax"], plan["bx"]
    ay, by = plan["ay"], plan["by"]
    az, bz = plan["az"], plan["bz"]
    ax_const = cfg["ax_const"]
    az_const = cfg["az_const"]
    ay_zero = cfg["ay_zero"]

    # ---- shared aux inputs (static per cfg+pose+quant grid: cached) ----
    static_key = (cfg["key"], pose.tobytes(), q_lo, q_scale)
    cached = _static_cache.get("k") == static_key
    if not cached:
        l16, l32 = _layouts(cfg)
        seg16 = {}
        for ri, (r0, Wr) in enumerate(r_chunks):
            seg16[f"ior{ri}"] = (r0 + np.arange(Wr)).astype(np.float16)
        if sgc is None:
            for ci, (c0, Wc) in enumerate(c_chunks):
                seg16[f"ioc{ci}"] = (c0 + np.arange(Wc)).astype(np.float16)
        else:
            Wcol = sgc["Wcol"]
            vals = (np.asarray(sgc["bases"], np.float32)[:, None]
                    + np.arange(Wcol, dtype=np.float32)[None, :]).astype(np.float16)
            seg16["iocf"] = vals.reshape(-1)
        seg32 = {"axr": ax, "azr": az, "ayr": ay}
        aux_inputs = {}
        aux_inputs["rowf16"] = np.concatenate(
            [seg16[name] for name, _ in l16])[None, :]
        if l32:
            aux_inputs["rowf32"] = np.concatenate(
                [seg32[name] for name, _ in l32])[None, :]
        Wr0 = r_chunks[0][1]
        pm = np.zeros((nb * Wr0, len(r_chunks) * (Wr0 + nb)), _dt)
        pidx = np.arange(nb * Wr0)
        for ri, (r0, Wr) in enumerate(r_chunks):
            base = ri * (Wr0 + nb)
            pm[pidx, base + pidx % Wr] = 1.0
            pm[pidx, base + Wr + pidx // Wr] = 1.0
        aux_inputs["pm"] = pm
        bcols_percore = []
        for g in range(N_CORES):
            tiles = active[g::N_CORES]
            bcols = np.zeros((P, 4 * n_t + 2), _dt)
            for k, t in enumerate(tiles):
                bcols[:, 4 * k + 0] = bx[t * P:(t + 1) * P]
                bcols[:, 4 * k + 1] = by[t * P:(t + 1) * P]
                bcols[:, 4 * k + 2] = bz[t * P:(t + 1) * P]
            bcols[:, 4 * n_t + 0] = q_scale
            bcols[:, 4 * n_t + 1] = q_lo
            bcols_percore.append(bcols)
        _static_cache.clear()
        _static_cache.update(k=static_key, aux=aux_inputs, bcols=bcols_percore)
    aux_inputs = _static_cache["aux"]
    bcols_percore = _static_cache["bcols"]

    # ---- per-core inputs: quantized depth slabs ----
    inv_scale = _dt(1.0 / q_scale)
    # round-half-up via +0.5 and truncation; values are >= 0 after -q_lo
    q_off = _dt(0.5 - q_lo / q_scale)
    need_clip = dmin < -CLAMP or dmax > CLAMP
    contig = active == list(range(active[0], active[0] + len(active))) \
        and len(active) == n_t * N_CORES
    if contig:
        # single vectorized quantization over the contiguous active block
        rows = depth[active[0] * P:(active[0] + len(active)) * P, :]
        if need_clip:
            rows = np.clip(rows, -CLAMP, CLAMP)
        qall = (rows * inv_scale + q_off).astype(np.uint16)
    in_maps = []
    for g in range(N_CORES):
        tiles = active[g::N_CORES]
        if contig:
            # tiles are active[0]+g, active[0]+g+8, ... -> strided view rows
            dslab = np.concatenate(
                [qall[(t - active[0]) * P:(t - active[0] + 1) * P, :]
                 for t in tiles], axis=0) if n_t > 1 else \
                qall[(tiles[0] - active[0]) * P:(tiles[0] - active[0] + 1) * P, :]
        else:
            dslab = np.zeros((n_t * P, W), np.uint16)
            for k, t in enumerate(tiles):
                rows = depth[t * P:(t + 1) * P, :]
                if need_clip:
                    rows = np.clip(rows, -CLAMP, CLAMP)
                dslab[k * P:(k + 1) * P, :] = \
                    (rows * inv_scale + q_off).astype(np.uint16)
        im = {"d1": dslab, "bcols": bcols_percore[g]}
        im.update(aux_inputs)
        in_maps.append(im)

    LAST_EXEC_NS["prep_wall"] = int((_time.perf_counter() - t_start) * 1e9)
    _t0 = _time.perf_counter()
    res = run_bass_kernel_spmd(nc, in_maps, core_ids=list(range(N_CORES)),
                               trace=TRACE)
    LAST_EXEC_NS["phase1_wall"] = int((_time.perf_counter() - _t0) * 1e9)
    if TRACE:
        LAST_EXEC_NS["phase1"] = res.exec_time_ns

    hist = np.zeros((M, M), _dt)
    for ri, (r0, Wr) in enumerate(r_chunks):
        for ci, (c0, Wc) in enumerate(c_chunks):
            tot = np.zeros((Wr, Wc), np.float64)
            for r in res.results:
                tot += r[f"win{ri}_{ci}"]
            rs = max(r0, 0)
            re = min(r0 + Wr, M)
            cs = max(c0, 0)
            ce = min(c0 + Wc, M)
            if rs < re and cs < ce:
                hist[rs:re, cs:ce] = tot[rs - r0:re - r0, cs - c0:ce - c0]

    if hist.sum() < 4096:
        return _host_reference(depth, pose)
    return hist.astype(_dt)


if __name__ == "__main__":
    rng = np.random.default_rng(0)
    d = rng.random((H, W), _dt)
    p = np.eye(4, dtype=_dt)
    out = kernel(d, p)
    print("sum", out.sum(), "nonzero", (out > 0).sum())


# revision 15
# speedup vs baseline: 1.1887x; 1.1110x over previous
"""Trainium2 Bass kernel for nn_DirectDepthMapper (histogram_binning).

Pipeline (matches reference.py):
  depth (H,W) -> per-pixel point (px,py,pz) -> pose transform -> masks ->
  (r,c) = round(g{z,x}/0.1 + 200) -> 400x400 histogram of valid points.

Strategy:
  - Scatter-add reformulated as windowed one-hot construction (DVE
    is_equal against iota rows, masked points pushed out of the window by
    arithmetic penalties) contracted on the TensorEngine:
    hist_win = sum_blocks ohR^T @ ohC accumulated in PSUM.
  - The active window (bounding box of reachable bins) is derived on the
    host from a clamped min/max of depth (cheap numpy) + interval
    arithmetic over the pose coefficients; row-tiles that cannot pass the
    height-band mask are skipped and the rest are balanced over 8 cores.
  - Depth ships as affine-quantized u16 (dequantized on device); all
    replicated constants (per-column coefficient rows, per-supergroup
    column iotas) ship as single rows and are partition-broadcast on
    device, so per-core transfer is ~0.5MB.
  - One SPMD call; each core emits its partial window histogram; the host
    sums 8 small windows into the 400x400 output.

Self-contained: hardcodes H=W=2048, 8 cores.
"""
import hashlib
import math
import os
import time as _time
from concurrent.futures import ThreadPoolExecutor

import numpy as np

import jax

# Persistent compilation cache: lets warm calls (and fresh processes on the
# same machine) skip the client-side BIR->NEFF recompile entirely.
try:
    jax.config.update("jax_compilation_cache_dir",
                      os.environ.get("BASS_JAX_CACHE_DIR", "/tmp/bass_jax_cache"))
    jax.config.update("jax_persistent_cache_min_compile_time_secs", 0.0)
    jax.config.update("jax_persistent_cache_min_entry_size_bytes", 0)
except Exception:
    pass

import concourse.bass as bass
import concourse.bacc as bacc
import concourse.mybir as mybir
import concourse.tile as tile
from concourse.bass_interp import get_hw_module
from concourse.bass_utils import run_bass_kernel_spmd

# In-process memo of the HLO->NEFF compile hook (pure function of the HLO
# bytes) as insurance for when the persistent cache misses.
try:
    import libneuronxla
    from concourse import bass2jax as _b2j
    _b2j.install_neuronx_cc_hook()
    if not getattr(libneuronxla, "_bass_memo_cc", None):
        _inner_cc = libneuronxla.neuronx_cc
        _cc_memo = {}

        def _memo_cc(code, code_format, platform_version, file_prefix):
            key = (hashlib.sha256(code).digest(), bytes(code_format),
                   str(platform_version))
            if key not in _cc_memo:
                _cc_memo[key] = _inner_cc(code, code_format, platform_version,
                                          file_prefix)
            return _cc_memo[key]

        libneuronxla.neuronx_cc = _memo_cc
        libneuronxla._bass_memo_cc = True
        _b2j.install_neuronx_cc_hook = lambda: None
except Exception:
    pass

# ---------------- problem constants (from reference.py) ----------------
H = W = 2048
N_CORES = 8
NEAR_TH = np.float32(0.1)
FAR_TH = np.float32(4.0)
H_MIN = np.float32(0.0)
H_MAX = np.float32(1.0)
CAMERA_HEIGHT = np.float32(0.0)
CELLS = int(math.ceil(40.0 / 0.1)) + 1   # 401
M = CELLS - 1                            # 400
SHIFT = math.floor(CELLS / 2.0)          # 200
MIN_PTS = 10

FX = np.float32(W / 2.0)
FY = np.float32(H / 2.0)
CX = int(FX) - 1
CY = int(FY) - 1

MAGIC = np.float32(1.5 * 2**23)          # fp32 round-to-nearest-int trick
CLAMP = 4.25                             # quantization clamp (> FAR_TH)
QLEV = 65535.0

# set by test harness for profiling; kernel() stores wall times here
TRACE = False
LAST_EXEC_NS = {}
P = 128                                  # partitions
ROW_TILES = H // P                       # 16
F32 = mybir.dt.float32
F16 = mybir.dt.float16
U16 = mybir.dt.uint16

_dt = np.float32


def _sxv():
    return ((np.arange(W, dtype=np.float64) - CX) / np.float64(FX)).astype(_dt)


def _syv():
    return ((np.arange(H, dtype=np.float64) - CY) / np.float64(FY)).astype(_dt)


# =====================================================================
# host-side interval arithmetic (plan the bin window + active tiles)
# =====================================================================
def _imul(a, b):
    c = [a[0] * b[0], a[0] * b[1], a[1] * b[0], a[1] * b[1]]
    return (min(c), max(c))


def _iadd(a, b):
    return (a[0] + b[0], a[1] + b[1])


def _coef_rows(pose, row):
    """a_i = pose[row,0]*sxv_i + pose[row,2]; b_j = pose[row,1]*syv_j"""
    p = np.asarray(pose, _dt)
    a = (p[row, 0] * _sxv() + p[row, 2]).astype(_dt)
    b = (p[row, 1] * _syv()).astype(_dt)
    k = float(p[row, 3])
    return a, b, k


def _valid_d(dlo, dhi):
    """hull of [dlo,dhi] restricted to the mask1-valid set |d| in [0.1, 4]."""
    lo, hi = None, None
    for a, b in ((-float(FAR_TH), -float(NEAR_TH)), (float(NEAR_TH), float(FAR_TH))):
        s, e = max(a, dlo), min(b, dhi)
        if s <= e:
            lo = s if lo is None else min(lo, s)
            hi = e if hi is None else max(hi, e)
    if lo is None:
        return None
    return (lo, hi)


def _plan(pose, dlo, dhi):
    d_int = _valid_d(dlo, dhi)
    if d_int is None:
        return None
    ax, bx, kx = _coef_rows(pose, 0)   # gx
    ay, by, ky = _coef_rows(pose, 1)   # gy raw
    az, bz, kz = _coef_rows(pose, 2)   # gz

    def box_for(a, b, k):
        c_int = _iadd((float(a.min()), float(a.max())),
                      (float(b.min()), float(b.max())))
        g = _iadd(_imul(d_int, c_int), (k, k))
        v = (10.0 * g[0] + SHIFT, 10.0 * g[1] + SHIFT)
        lo = int(np.floor(v[0])) - 1
        hi = int(np.ceil(v[1])) + 1
        return max(lo, -1), min(hi, M)

    rbox = box_for(az, bz, kz)
    cbox = box_for(ax, bx, kx)
    if rbox[0] > rbox[1] or cbox[0] > cbox[1]:
        return None

    u_hi = float(CAMERA_HEIGHT - ky - H_MIN)   # valid iff u_lo < w < u_hi
    u_lo = float(CAMERA_HEIGHT - ky - H_MAX)
    a_int = (float(ay.min()), float(ay.max()))
    active = []
    for t in range(ROW_TILES):
        bt = by[t * P:(t + 1) * P]
        c_int = _iadd(a_int, (float(bt.min()), float(bt.max())))
        w_int = _imul(d_int, c_int)
        if w_int[0] < u_hi and w_int[1] > u_lo:
            active.append(t)
    return dict(rbox=rbox, cbox=cbox, active=active,
                ax=ax, bx=bx, kx=kx, ay=ay, by=by, ky=ky,
                az=az, bz=bz, kz=kz, u_lo=u_lo, u_hi=u_hi)


def _pad_to(x, m):
    return ((x + m - 1) // m) * m


def _chunks(lo, hi, cap):
    out = []
    x = lo
    while x <= hi:
        wdt = min(cap, hi - x + 1)
        out.append((x, wdt))
        x += wdt
    return out


# =====================================================================
# phase 1 kernel builder
# =====================================================================
_phase1_cache = {}


def _layouts(cfg):
    """Segment layouts of the merged replicated-row inputs."""
    l16 = []
    for ri, (r0, Wr) in enumerate(cfg["r_chunks"]):
        l16.append((f"ior{ri}", Wr))
    if cfg["sgc"] is None:
        for ci, (c0, Wc) in enumerate(cfg["c_chunks"]):
            l16.append((f"ioc{ci}", Wc))
    else:
        l16.append(("iocf", (W // cfg["nb"]) * cfg["sgc"]["Wcol"]))
    l32 = []
    if cfg["ax_const"] is None:
        l32.append(("axr", W))
    if cfg["az_const"] is None:
        l32.append(("azr", W))
    if not cfg["ay_zero"]:
        l32.append(("ayr", W))
    return l16, l32


def _build_phase1(cfg):
    key = cfg["key"]
    if key in _phase1_cache:
        return _phase1_cache[key]

    n_t = cfg["n_t"]
    nb = cfg["nb"]
    r_chunks = cfg["r_chunks"]      # list of (r0, Wr)
    c_chunks = cfg["c_chunks"]      # list of (c0, Wc)
    ax_const = cfg["ax_const"]      # float or None
    az_const = cfg["az_const"]
    bx_zero = cfg["bx_zero"]
    bz_zero = cfg["bz_zero"]
    ay_zero = cfg["ay_zero"]
    kx = cfg["kx"]
    kz = cfg["kz"]
    u_lo = cfg["u_lo"]
    u_hi = cfg["u_hi"]
    sgc = cfg.get("sgc")          # per-supergroup c windows: (Wcol, bases)

    nc = bacc.Bacc("TRN2", target_bir_lowering=False, debug=False,
                   num_devices=N_CORES)
    d_dram = nc.dram_tensor("d1", [n_t * P, W], U16, kind="ExternalInput").ap()
    # per-row (partition) coefficient columns, packed [P, 4*n_t + 2]
    # (bx, by, bz, 0) per tile, then (qscale, qlo)
    b_dram = nc.dram_tensor("bcols", [P, 4 * n_t + 2], F32,
                            kind="ExternalInput").ap()
    need_ax = ax_const is None
    need_az = az_const is None
    need_ay = not ay_zero
    # merged row tensors: one f16 row (r/c iotas), one f32 row (coef rows)
    l16, l32 = _layouts(cfg)
    L16 = sum(n for _, n in l16)
    L32 = sum(n for _, n in l32)
    rowf16_dram = nc.dram_tensor("rowf16", [1, L16], F16,
                                 kind="ExternalInput").ap()
    if L32:
        rowf32_dram = nc.dram_tensor("rowf32", [1, L32], F32,
                                     kind="ExternalInput").ap()
    # merged per-partition constants: [sel_ri | m8_ri] per r chunk
    Wr0 = r_chunks[0][1]
    pm_dram = nc.dram_tensor("pm", [nb * Wr0, len(r_chunks) * (Wr0 + nb)], F32,
                             kind="ExternalInput").ap()
    win_dram = {}
    for ri, (r0, Wr) in enumerate(r_chunks):
        for ci, (c0, Wc) in enumerate(c_chunks):
            win_dram[(ri, ci)] = nc.dram_tensor(
                f"win{ri}_{ci}", [Wr, Wc], F32, kind="ExternalOutput").ap()

    A = mybir.AluOpType
    SENT_LO = float(min(r0 for r0, _ in r_chunks) - 5)
    SENT_HI = float(max(r0 + w for r0, w in r_chunks) + 4)
    PEN = 256.0  # > sentinel span (Wr+9 <= 137), 4*PEN + |SENT| < 2048 (f16 int-exact)

    with tile.TileContext(nc) as tc:
        with tc.tile_pool(name="const", bufs=1) as cpool, \
             tc.tile_pool(name="sbuf", bufs=2) as pool, \
             tc.tile_pool(name="oh", bufs=2) as ohpool, \
             tc.tile_pool(name="psum", bufs=1, space="PSUM") as psum_pool, \
             tc.tile_pool(name="psum2", bufs=2, space="PSUM") as psum2_pool:

            # ---- constants (two broadcasts cover every replicated row) ----
            r16a = cpool.tile([1, L16], F16, tag="r16a")
            nc.sync.dma_start(out=r16a, in_=rowf16_dram)
            r16 = cpool.tile([P, L16], F16, tag="r16")
            nc.gpsimd.partition_broadcast(r16, r16a)
            if L32:
                r32a = cpool.tile([1, L32], F32, tag="r32a")
                nc.sync.dma_start(out=r32a, in_=rowf32_dram)
                r32 = cpool.tile([P, L32], F32, tag="r32")
                nc.gpsimd.partition_broadcast(r32, r32a)
            seg16 = {}
            off = 0
            for name, ln in l16:
                seg16[name] = r16[:, off:off + ln]
                off += ln
            seg32 = {}
            off = 0
            for name, ln in l32:
                seg32[name] = r32[:, off:off + ln]
                off += ln
            ior = {ri: seg16[f"ior{ri}"] for ri, _ in enumerate(r_chunks)}
            if sgc is None:
                ioc = {ci: seg16[f"ioc{ci}"] for ci, _ in enumerate(c_chunks)}
            else:
                iocf = seg16["iocf"]
                zlh = cpool.tile([P, nb * r_chunks[0][1]], F16, tag="zlh")
                nc.vector.memset(zlh, 0.0)
                zrh = cpool.tile([P, nb * c_chunks[0][1]], F16, tag="zrh")
                nc.vector.memset(zrh, 0.0)
            if need_ax:
                ax_t = seg32["axr"]
            if need_az:
                az_t = seg32["azr"]
            if need_ay:
                ay_t = seg32["ayr"]
            pm = cpool.tile([nb * Wr0, len(r_chunks) * (Wr0 + nb)], F32,
                            tag="pm")
            nc.sync.dma_start(out=pm, in_=pm_dram)
            sel = {}
            m8 = {}
            for ri, (r0, Wr) in enumerate(r_chunks):
                base = ri * (Wr0 + nb)
                sel[ri] = pm[:, base:base + Wr]
                m8[ri] = pm[:, base + Wr:base + Wr + nb]
            bcols = cpool.tile([P, 4 * n_t + 2], F32, tag="bcols")
            nc.sync.dma_start(out=bcols, in_=b_dram)
            qs_ap = bcols[:, 4 * n_t + 0:4 * n_t + 1]
            ql_ap = bcols[:, 4 * n_t + 1:4 * n_t + 2]

            psum = {}
            for ri, (r0, Wr) in enumerate(r_chunks):
                for ci, (c0, Wc) in enumerate(c_chunks):
                    psum[(ri, ci)] = psum_pool.tile([nb * Wr, nb * Wc], F32,
                                                    tag=f"ps{ri}_{ci}",
                                                    name=f"ps{ri}_{ci}")

            n_super = W // nb
            if sgc is not None:
                for ri, (r0, Wr) in enumerate(r_chunks):
                    nc.tensor.matmul(psum[(ri, 0)], zlh, zrh,
                                     start=True, stop=False)
            CH = 1024                     # column chunk for pipelining
            n_cc = W // CH
            sg_per_cc = CH // nb
            for t in range(n_t):
                q = pool.tile([P, W], U16, tag="q")
                nc.sync.dma_start(out=q, in_=d_dram[t * P:(t + 1) * P, :])
                # dequant: d = qscale*q + qlo
                d = pool.tile([P, W], F32, tag="d")
                nc.scalar.activation(out=d, in_=q,
                                     func=mybir.ActivationFunctionType.Copy,
                                     bias=0.0, scale=qs_ap)
                nc.vector.tensor_scalar(out=d, in0=d, scalar1=ql_ap,
                                        scalar2=None, op0=A.add)
                bx_ap = bcols[:, 4 * t + 0:4 * t + 1]
                by_ap = bcols[:, 4 * t + 1:4 * t + 2]
                bz_ap = bcols[:, 4 * t + 2:4 * t + 3]

                for cc in range(n_cc):
                    csl = slice(cc * CH, (cc + 1) * CH)
                    dC = d[:, csl]

                    # ---- c index ----
                    vc = pool.tile([P, CH], F32, tag="vc")
                    if ax_const is None:
                        tC = pool.tile([P, CH], F32, tag="tC")
                        nc.vector.tensor_tensor(out=tC, in0=dC,
                                                in1=ax_t[:, csl], op=A.mult)
                        if not bx_zero:
                            nc.vector.scalar_tensor_tensor(
                                out=tC, in0=dC, scalar=bx_ap, in1=tC,
                                op0=A.mult, op1=A.add)
                        nc.vector.tensor_scalar(
                            out=vc, in0=tC, scalar1=10.0,
                            scalar2=float(SHIFT + 10.0 * kx),
                            op0=A.mult, op1=A.add)
                    else:
                        if not bx_zero:
                            tC = pool.tile([P, CH], F32, tag="tC")
                            nc.vector.tensor_scalar(out=tC, in0=dC, scalar1=bx_ap,
                                                    scalar2=None, op0=A.mult)
                            nc.vector.scalar_tensor_tensor(
                                out=tC, in0=dC, scalar=float(ax_const), in1=tC,
                                op0=A.mult, op1=A.add)
                            nc.vector.tensor_scalar(
                                out=vc, in0=tC, scalar1=10.0,
                                scalar2=float(SHIFT + 10.0 * kx),
                                op0=A.mult, op1=A.add)
                        else:
                            nc.vector.tensor_scalar(
                                out=vc, in0=dC, scalar1=float(10.0 * ax_const),
                                scalar2=float(SHIFT + 10.0 * kx),
                                op0=A.mult, op1=A.add)
                    vcM = pool.tile([P, CH], F32, tag="vcM")
                    nc.scalar.activation(out=vcM, in_=vc,
                                         func=mybir.ActivationFunctionType.Copy,
                                         bias=float(MAGIC))
                    vc16 = pool.tile([P, CH], F16, tag="vc16")
                    nc.scalar.activation(out=vc16, in_=vcM,
                                         func=mybir.ActivationFunctionType.Copy,
                                         bias=float(-MAGIC))

                    # ---- r index ----
                    vr = pool.tile([P, CH], F32, tag="vr")
                    if az_const is None:
                        tZ = pool.tile([P, CH], F32, tag="tZ")
                        nc.vector.tensor_tensor(out=tZ, in0=dC,
                                                in1=az_t[:, csl], op=A.mult)
                        if not bz_zero:
                            nc.vector.scalar_tensor_tensor(
                                out=tZ, in0=dC, scalar=bz_ap, in1=tZ,
                                op0=A.mult, op1=A.add)
                        nc.vector.tensor_scalar(
                            out=vr, in0=tZ, scalar1=10.0,
                            scalar2=float(SHIFT + 10.0 * kz),
                            op0=A.mult, op1=A.add)
                    else:
                        if not bz_zero:
                            tZ = pool.tile([P, CH], F32, tag="tZ")
                            nc.vector.tensor_scalar(out=tZ, in0=dC, scalar1=bz_ap,
                                                    scalar2=None, op0=A.mult)
                            nc.vector.scalar_tensor_tensor(
                                out=tZ, in0=dC, scalar=float(az_const), in1=tZ,
                                op0=A.mult, op1=A.add)
                            nc.vector.tensor_scalar(
                                out=vr, in0=tZ, scalar1=10.0,
                                scalar2=float(SHIFT + 10.0 * kz),
                                op0=A.mult, op1=A.add)
                        else:
                            nc.vector.tensor_scalar(
                                out=vr, in0=dC, scalar1=float(10.0 * az_const),
                                scalar2=float(SHIFT + 10.0 * kz),
                                op0=A.mult, op1=A.add)
                    vrM = pool.tile([P, CH], F32, tag="vrM")
                    nc.scalar.activation(out=vrM, in_=vr,
                                         func=mybir.ActivationFunctionType.Copy,
                                         bias=float(MAGIC))
                    vr16 = pool.tile([P, CH], F16, tag="vr16")
                    nc.scalar.activation(out=vr16, in_=vrM,
                                         func=mybir.ActivationFunctionType.Copy,
                                         bias=float(-MAGIC))
                    # clamp to sentinels FIRST, then add penalties (PEN >
                    # sentinel span) -- keeps every value f16-int-exact and
                    # guarantees masked points never collide with the window.
                    nc.vector.tensor_scalar(out=vr16, in0=vr16, scalar1=SENT_HI,
                                            scalar2=SENT_LO, op0=A.min, op1=A.max)

                    # ---- masks -> penalties on vr16 ----
                    wY = pool.tile([P, CH], F32, tag="wY")
                    if need_ay:
                        nc.vector.tensor_tensor(out=wY, in0=dC,
                                                in1=ay_t[:, csl], op=A.mult)
                        nc.vector.scalar_tensor_tensor(
                            out=wY, in0=dC, scalar=by_ap, in1=wY,
                            op0=A.mult, op1=A.add)
                    else:
                        nc.scalar.activation(out=wY, in_=dC,
                                             func=mybir.ActivationFunctionType.Copy,
                                             bias=0.0, scale=by_ap)
                    vio = pool.tile([P, CH], F16, tag="vio")
                    ad = pool.tile([P, CH], F32, tag="ad")
                    nc.scalar.activation(out=ad, in_=dC,
                                         func=mybir.ActivationFunctionType.Abs)
                    for src_t, thr, cmp in ((wY, float(u_hi), A.is_ge),
                                            (wY, float(u_lo), A.is_le),
                                            (ad, float(NEAR_TH), A.is_lt),
                                            (ad, float(FAR_TH), A.is_ge)):
                        nc.vector.tensor_scalar(out=vio, in0=src_t, scalar1=thr,
                                                scalar2=PEN, op0=cmp, op1=A.mult)
                        nc.vector.tensor_tensor(out=vr16, in0=vr16, in1=vio,
                                                op=A.add)

                    # ---- one-hot + matmul accumulate ----
                    G = 32
                    n_groups = sg_per_cc // G
                    for g2 in range(n_groups):
                        sl = slice(g2 * G * nb, (g2 + 1) * G * nb)
                        lhsT = {}
                        for ri, (r0, Wr) in enumerate(r_chunks):
                            lt = ohpool.tile([P, G * nb * Wr], F16,
                                             tag=f"lh{ri}", name=f"lh{ri}")
                            nc.vector.tensor_tensor(
                                out=lt.rearrange("p (n w) -> p n w", w=Wr),
                                in0=vr16[:, sl][:, :, None].broadcast_to([P, G * nb, Wr]),
                                in1=ior[ri][:, None, :].broadcast_to([P, G * nb, Wr]),
                                op=A.is_equal)
                            lhsT[ri] = lt
                        rhs = {}
                        if sgc is None:
                            for ci, (c0, Wc) in enumerate(c_chunks):
                                rh = ohpool.tile([P, G * nb * Wc], F16,
                                                 tag=f"rh{ci}", name=f"rh{ci}")
                                nc.vector.tensor_tensor(
                                    out=rh.rearrange("p (n w) -> p n w", w=Wc),
                                    in0=vc16[:, sl][:, :, None].broadcast_to([P, G * nb, Wc]),
                                    in1=ioc[ci][:, None, :].broadcast_to([P, G * nb, Wc]),
                                    op=A.is_equal)
                                rhs[ci] = rh
                        else:
                            WCOL = sgc["Wcol"]
                            s_base = cc * sg_per_cc + g2 * G
                            rh = ohpool.tile([P, G * nb * WCOL], F16,
                                             tag="rh0", name="rh0")
                            vcv = vc16[:, sl].rearrange("p (g n) -> p g n", g=G)
                            iov = iocf[:, s_base * WCOL:(s_base + G) * WCOL] \
                                .rearrange("p (g w) -> p g w", g=G)
                            nc.vector.tensor_tensor(
                                out=rh.rearrange("p (g n w) -> p g n w", g=G, w=WCOL),
                                in0=vcv[:, :, :, None].broadcast_to([P, G, nb, WCOL]),
                                in1=iov[:, :, None, :].broadcast_to([P, G, nb, WCOL]),
                                op=A.is_equal)
                            rhs[0] = rh
                        for k in range(G):
                            s = cc * sg_per_cc + g2 * G + k
                            last = (t == n_t - 1) and (s == n_super - 1)
                            for ci, (c0, Wc) in enumerate(c_chunks):
                                for ri, (r0, Wr) in enumerate(r_chunks):
                                    if sgc is None:
                                        nc.tensor.matmul(
                                            psum[(ri, ci)],
                                            lhsT[ri][:, k * nb * Wr:(k + 1) * nb * Wr],
                                            rhs[ci][:, k * nb * Wc:(k + 1) * nb * Wc],
                                            start=(s == 0 and t == 0),
                                            stop=last)
                                    else:
                                        WCOL = sgc["Wcol"]
                                        o_s = sgc["bases"][s] - c0
                                        out_ap = psum[(ri, ci)].rearrange(
                                            "m (n q) -> m n q", q=Wc)[:, :, o_s:o_s + WCOL]
                                        nc.tensor.matmul(
                                            out_ap,
                                            lhsT[ri][:, k * nb * Wr:(k + 1) * nb * Wr],
                                            rhs[ci][:, k * nb * WCOL:(k + 1) * nb * WCOL],
                                            start=False,
                                            stop=last)
            # ---- extract: cross-block fold ----
            for ri, (r0, Wr) in enumerate(r_chunks):
                for ci, (c0, Wc) in enumerate(c_chunks):
                    psb = pool.tile([nb * Wr, nb * Wc], F32, tag="psb")
                    nc.vector.tensor_tensor(
                        out=psb.rearrange("p (n w) -> p n w", n=nb),
                        in0=psum[(ri, ci)].rearrange("p (n w) -> p n w", n=nb),
                        in1=m8[ri][:, :, None].broadcast_to([nb * Wr, nb, Wc]),
                        op=A.mult)
                    ps2 = psum2_pool.tile([Wr, nb * Wc], F32, tag="ps2")
                    nc.tensor.matmul(ps2, sel[ri], psb, start=True, stop=True)
                    o2 = pool.tile([Wr, nb * Wc], F32, tag="o2")
                    nc.vector.tensor_copy(out=o2, in_=ps2)
                    acc = pool.tile([Wr, Wc], F32, tag="acc")
                    nc.vector.tensor_copy(out=acc, in_=o2[:, 0:Wc])
                    for b in range(1, nb):
                        nc.vector.tensor_tensor(out=acc, in0=acc,
                                                in1=o2[:, b * Wc:(b + 1) * Wc],
                                                op=A.add)
                    nc.sync.dma_start(out=win_dram[(ri, ci)], in_=acc)

    nc.compile()
    nc.m = get_hw_module(nc.m)
    _phase1_cache[key] = nc
    return nc


# =====================================================================
# host fallback (exact reference replication, used for gate corner cases)
# =====================================================================
def _host_reference(depth, pose):
    d = np.asarray(depth, _dt)
    pose = np.asarray(pose, _dt)
    sx = _sxv()
    sy = _syv()
    px = d * sx[None, :]
    py = d * sy[:, None]
    pz = d
    mask1 = (np.abs(pz) < FAR_TH) & (np.abs(pz) >= NEAR_TH)
    ones = np.ones_like(d)
    gx = pose[0, 0] * px + pose[0, 1] * py + pose[0, 2] * pz + pose[0, 3] * ones
    gy = pose[1, 0] * px + pose[1, 1] * py + pose[1, 2] * pz + pose[1, 3] * ones
    gz = pose[2, 0] * px + pose[2, 1] * py + pose[2, 2] * pz + pose[2, 3] * ones
    gy = -gy + CAMERA_HEIGHT
    mask2 = mask1 & (gy > H_MIN) & (gy < H_MAX)
    r = np.round(gz / _dt(0.1) + _dt(SHIFT)).astype(np.int64)
    c = np.round(gx / _dt(0.1) + _dt(SHIFT)).astype(np.int64)
    inb = (r >= 0) & (r < M) & (c >= 0) & (c < M)
    valid = mask2 & inb
    flat = np.where(valid, r * M + c, 0)
    hist = np.bincount(flat.ravel(), weights=valid.ravel().astype(np.float64),
                       minlength=M * M).astype(_dt).reshape(M, M)
    n1 = int(mask1.sum())
    n2 = int(mask2.sum())
    ok = (n1 >= 20) and (n2 > MIN_PTS)
    return hist if ok else np.zeros((M, M), _dt)


# =====================================================================
# main entry
# =====================================================================
_static_cache = {}


def _make_cfg(plan, dlo, dhi):
    r_lo, r_hi = plan["rbox"]
    c_lo, c_hi = plan["cbox"]
    boxw_r = r_hi - r_lo + 1
    boxw_c = c_hi - c_lo + 1

    Wr_u = min(128, _pad_to(boxw_r, 2))
    nb = 1
    while nb < 8 and 2 * nb * Wr_u <= P:
        nb *= 2
    r_chunks = _chunks(r_lo, r_hi, Wr_u)
    r_chunks = [(r0, Wr_u) for (r0, w) in r_chunks]
    c_cap = (512 // nb) & ~1
    c_chunks = _chunks(c_lo, c_hi, c_cap)
    c_chunks = [(c0, _pad_to(w, 2)) for (c0, w) in c_chunks]
    assert len(r_chunks) * len(c_chunks) <= 6, "window too large for PSUM"

    sgc = None
    if len(c_chunks) == 1:
        n_super_all = W // nb
        ax_v, bx_v = plan["ax"], plan["bx"]
        kx_v = plan["kx"]
        bxa = np.concatenate([bx_v[t * P:(t + 1) * P] for t in plan["active"]]) \
            if plan["active"] else bx_v
        bx_int = (float(bxa.min()), float(bxa.max()))
        d_int = _valid_d(dlo, dhi)
        bases = []
        tops = []
        for s in range(n_super_all):
            ag = ax_v[s * nb:(s + 1) * nb]
            ci_ = _iadd((float(ag.min()), float(ag.max())), bx_int)
            g = _iadd(_imul(d_int, ci_), (kx_v, kx_v))
            v = (10.0 * g[0] + SHIFT, 10.0 * g[1] + SHIFT)
            bases.append(max(int(np.floor(v[0])) - 1, c_lo))
            tops.append(min(int(np.ceil(v[1])) + 1, c_lo + c_chunks[0][1] - 1))
        Wcol = _pad_to(max(t - b + 1 for b, t in zip(bases, tops)), 2)
        bases = [min(b, c_lo + c_chunks[0][1] - Wcol) for b in bases]
        if Wcol + 4 < c_chunks[0][1]:
            sgc = dict(Wcol=Wcol, bases=tuple(bases))

    active = plan["active"]
    n_t = (len(active) + N_CORES - 1) // N_CORES

    ax, bx = plan["ax"], plan["bx"]
    ay, by = plan["ay"], plan["by"]
    az, bz = plan["az"], plan["bz"]
    ax_const = float(ax[0]) if np.all(ax == ax[0]) else None
    az_const = float(az[0]) if np.all(az == az[0]) else None
    bx_zero = bool(np.all(bx == 0))
    bz_zero = bool(np.all(bz == 0))
    ay_zero = bool(np.all(ay == 0))

    cfg = dict(
        key=(n_t, nb, tuple(r_chunks), tuple(c_chunks),
             ax_const, az_const, bx_zero, bz_zero, ay_zero,
             plan["kx"], plan["kz"], plan["u_lo"], plan["u_hi"],
             (sgc["Wcol"], sgc["bases"]) if sgc else None),
        n_t=n_t, nb=nb, r_chunks=r_chunks, c_chunks=c_chunks,
        ax_const=ax_const, az_const=az_const,
        bx_zero=bx_zero, bz_zero=bz_zero, ay_zero=ay_zero,
        kx=plan["kx"], kz=plan["kz"], u_lo=plan["u_lo"], u_hi=plan["u_hi"],
        sgc=sgc)
    return cfg


_pool = ThreadPoolExecutor(max_workers=8)
_opt_cache = {}   # pose -> (plan, cfg) for the optimistic [0,1) grid


def _is_contig(active, n_t):
    return (active == list(range(active[0], active[0] + len(active)))
            and len(active) == n_t * N_CORES)


def kernel(depth, pose):
    t_start = _time.perf_counter()
    depth = np.asarray(depth, _dt)
    pose = np.asarray(pose, _dt)
    assert depth.shape == (H, W)

    # ---- kick off the min/max verification scan on worker threads ----
    step = H // 4
    mm_futs = [_pool.submit(np.min, depth[i:i + step])
               for i in range(0, H, step)] + \
              [_pool.submit(np.max, depth[i:i + step])
               for i in range(0, H, step)]

    # ---- optimistic path: assume depth within [0, 1] ----
    qs1 = 1.0 / QLEV
    pk = pose.tobytes()
    if _opt_cache.get("pose") != pk:
        p_o = _plan(pose, 0.0 - qs1, 1.0 + qs1)
        c_o = _make_cfg(p_o, 0.0 - qs1, 1.0 + qs1) \
            if (p_o is not None and p_o["active"]) else None
        _opt_cache.clear()
        _opt_cache.update(pose=pk, plan=p_o, cfg=c_o)
    plan_o, cfg_o = _opt_cache["plan"], _opt_cache["cfg"]
    qall = None
    q_futs = []
    contig_o = cfg_o is not None and _is_contig(plan_o["active"], cfg_o["n_t"])
    if contig_o:
        a0 = plan_o["active"][0]
        rows_o = depth[a0 * P:(a0 + len(plan_o["active"])) * P, :]
        qall_o = np.empty(rows_o.shape, np.uint16)
        lev = _dt(QLEV)
        half = _dt(0.5)

        def _qo(i0, i1):
            np.copyto(qall_o[i0:i1], rows_o[i0:i1] * lev + half,
                      casting='unsafe')
        qstep = (rows_o.shape[0] + 3) // 4
        q_futs = [_pool.submit(_qo, i, min(i + qstep, rows_o.shape[0]))
                  for i in range(0, rows_o.shape[0], qstep)]

    dmin = float(min(f.result() for f in mm_futs[:4]))
    dmax = float(max(f.result() for f in mm_futs[4:]))

    if contig_o and 0.0 <= dmin and dmax <= 1.0:
        for f in q_futs:
            f.result()
        plan, cfg = plan_o, cfg_o
        q_lo, q_hi = 0.0, 1.0
        q_scale = qs1
        qall = qall_o
    else:
        for f in q_futs:
            f.result()      # drain before discarding the buffer
        # quantization domain, snapped to a coarse grid for config stability
        q_lo = max(math.floor(dmin * 16.0) / 16.0, -CLAMP)
        q_hi = min(math.ceil(dmax * 16.0) / 16.0, CLAMP)
        if q_hi <= q_lo:
            q_hi = q_lo + 1.0 / 16.0
        q_scale = (q_hi - q_lo) / QLEV
        # plan over the clamped range, padded by one quantization step
        dlo = max(dmin, -float(FAR_TH)) - q_scale
        dhi = min(dmax, float(FAR_TH)) + q_scale
        plan = _plan(pose, dlo, dhi)
        if plan is None or not plan["active"]:
            return _host_reference(depth, pose)
        cfg = _make_cfg(plan, dlo, dhi)

    # Padded slab rows (when active tiles don't divide evenly) carry q=0,
    # which dequantizes to d=q_lo with by=0. Verify such rows are always
    # masked (near/far or height band); else fall back to the exact host path.
    n_fill = cfg["n_t"] * N_CORES - len(plan["active"])
    if n_fill > 0:
        d_f = q_lo
        safe = abs(d_f) < float(NEAR_TH) or abs(d_f) >= float(FAR_TH)
        if not safe:
            ay_v = plan["ay"]
            w_lo = min(d_f * float(ay_v.min()), d_f * float(ay_v.max()))
            w_hi = max(d_f * float(ay_v.min()), d_f * float(ay_v.max()))
            safe = (w_hi <= plan["u_lo"]) or (w_lo >= plan["u_hi"])
        if not safe:
            return _host_reference(depth, pose)

    nc = _build_phase1(cfg)

    r_chunks = cfg["r_chunks"]
    c_chunks = cfg["c_chunks"]
    nb = cfg["nb"]
    n_t = cfg["n_t"]
    sgc = cfg["sgc"]
    active = plan["active"]
    ax, bx = plan["ax"], plan["bx"]
    ay, by = plan["ay"], plan["by"]
    az, bz = plan["az"], plan["bz"]
    ax_const = cfg["ax_const"]
    az_const = cfg["az_const"]
    ay_zero = cfg["ay_zero"]

    # ---- shared aux inputs (static per cfg+pose+quant grid: cached) ----
    static_key = (cfg["key"], pose.tobytes(), q_lo, q_scale)
    cached = _static_cache.get("k") == static_key
    if not cached:
        l16, l32 = _layouts(cfg)
        seg16 = {}
        for ri, (r0, Wr) in enumerate(r_chunks):
            seg16[f"ior{ri}"] = (r0 + np.arange(Wr)).astype(np.float16)
        if sgc is None:
            for ci, (c0, Wc) in enumerate(c_chunks):
                seg16[f"ioc{ci}"] = (c0 + np.arange(Wc)).astype(np.float16)
        else:
            Wcol = sgc["Wcol"]
            vals = (np.asarray(sgc["bases"], np.float32)[:, None]
                    + np.arange(Wcol, dtype=np.float32)[None, :]).astype(np.float16)
            seg16["iocf"] = vals.reshape(-1)
        seg32 = {"axr": ax, "azr": az, "ayr": ay}
        aux_inputs = {}
        aux_inputs["rowf16"] = np.concatenate(
            [seg16[name] for name, _ in l16])[None, :]
        if l32:
            aux_inputs["rowf32"] = np.concatenate(
                [seg32[name] for name, _ in l32])[None, :]
        Wr0 = r_chunks[0][1]
        pm = np.zeros((nb * Wr0, len(r_chunks) * (Wr0 + nb)), _dt)
        pidx = np.arange(nb * Wr0)
        for ri, (r0, Wr) in enumerate(r_chunks):
            base = ri * (Wr0 + nb)
            pm[pidx, base + pidx % Wr] = 1.0
            pm[pidx, base + Wr + pidx // Wr] = 1.0
        aux_inputs["pm"] = pm
        bcols_percore = []
        for g in range(N_CORES):
            tiles = active[g::N_CORES]
            bcols = np.zeros((P, 4 * n_t + 2), _dt)
            for k, t in enumerate(tiles):
                bcols[:, 4 * k + 0] = bx[t * P:(t + 1) * P]
                bcols[:, 4 * k + 1] = by[t * P:(t + 1) * P]
                bcols[:, 4 * k + 2] = bz[t * P:(t + 1) * P]
            bcols[:, 4 * n_t + 0] = q_scale
            bcols[:, 4 * n_t + 1] = q_lo
            bcols_percore.append(bcols)
        _static_cache.clear()
        _static_cache.update(k=static_key, aux=aux_inputs, bcols=bcols_percore)
    aux_inputs = _static_cache["aux"]
    bcols_percore = _static_cache["bcols"]

    # ---- per-core inputs: quantized depth slabs ----
    inv_scale = _dt(1.0 / q_scale)
    # round-half-up via +0.5 and truncation; values are >= 0 after -q_lo
    q_off = _dt(0.5 - q_lo / q_scale)
    need_clip = dmin < -CLAMP or dmax > CLAMP
    contig = qall is not None or _is_contig(active, n_t)
    if contig and qall is None:
        # vectorized quantization over the contiguous active block,
        # 4 row-stripes in parallel
        rows = depth[active[0] * P:(active[0] + len(active)) * P, :]
        if need_clip:
            rows = np.clip(rows, -CLAMP, CLAMP)
        qall = np.empty(rows.shape, np.uint16)

        def _qstripe(i0, i1):
            np.copyto(qall[i0:i1], rows[i0:i1] * inv_scale + q_off,
                      casting='unsafe')
        n_rows = rows.shape[0]
        step = (n_rows + 3) // 4
        list(_pool.map(lambda i: _qstripe(i, min(i + step, n_rows)),
                       range(0, n_rows, step)))
    in_maps = []
    for g in range(N_CORES):
        tiles = active[g::N_CORES]
        if contig:
            # tiles are active[0]+g, active[0]+g+8, ... -> strided view rows
            dslab = np.concatenate(
                [qall[(t - active[0]) * P:(t - active[0] + 1) * P, :]
                 for t in tiles], axis=0) if n_t > 1 else \
                qall[(tiles[0] - active[0]) * P:(tiles[0] - active[0] + 1) * P, :]
        else:
            dslab = np.zeros((n_t * P, W), np.uint16)
            for k, t in enumerate(tiles):
                rows = depth[t * P:(t + 1) * P, :]
                if need_clip:
                    rows = np.clip(rows, -CLAMP, CLAMP)
                dslab[k * P:(k + 1) * P, :] = \
                    (rows * inv_scale + q_off).astype(np.uint16)
        im = {"d1": dslab, "bcols": bcols_percore[g]}
        im.update(aux_inputs)
        in_maps.append(im)

    LAST_EXEC_NS["prep_wall"] = int((_time.perf_counter() - t_start) * 1e9)
    _t0 = _time.perf_counter()
    res = run_bass_kernel_spmd(nc, in_maps, core_ids=list(range(N_CORES)),
                               trace=TRACE)
    LAST_EXEC_NS["phase1_wall"] = int((_time.perf_counter() - _t0) * 1e9)
    if TRACE:
        LAST_EXEC_NS["phase1"] = res.exec_time_ns

    hist = np.zeros((M, M), _dt)
    for ri, (r0, Wr) in enumerate(r_chunks):
        for ci, (c0, Wc) in enumerate(c_chunks):
            tot = np.zeros((Wr, Wc), np.float64)
            for r in res.results:
                tot += r[f"win{ri}_{ci}"]
            rs = max(r0, 0)
            re = min(r0 + Wr, M)
            cs = max(c0, 0)
            ce = min(c0 + Wc, M)
            if rs < re and cs < ce:
                hist[rs:re, cs:ce] = tot[rs - r0:re - r0, cs - c0:ce - c0]

    if hist.sum() < 4096:
        return _host_reference(depth, pose)
    return hist.astype(_dt)


if __name__ == "__main__":
    rng = np.random.default_rng(0)
    d = rng.random((H, W), _dt)
    p = np.eye(4, dtype=_dt)
    out = kernel(d, p)
    print("sum", out.sum(), "nonzero", (out > 0).sum())


# revision 20
# speedup vs baseline: 1.2560x; 1.0566x over previous
"""Trainium2 Bass kernel for nn_DirectDepthMapper (histogram_binning).

Pipeline (matches reference.py):
  depth (H,W) -> per-pixel point (px,py,pz) -> pose transform -> masks ->
  (r,c) = round(g{z,x}/0.1 + 200) -> 400x400 histogram of valid points.

Strategy:
  - Scatter-add reformulated as windowed one-hot construction (DVE
    is_equal against iota rows, masked points pushed out of the window by
    arithmetic penalties) contracted on the TensorEngine:
    hist_win = sum_blocks ohR^T @ ohC accumulated in PSUM.
  - The active window (bounding box of reachable bins) is derived on the
    host from a clamped min/max of depth (cheap numpy) + interval
    arithmetic over the pose coefficients; row-tiles that cannot pass the
    height-band mask are skipped and the rest are balanced over 8 cores.
  - Depth ships as affine-quantized u16 (dequantized on device); all
    replicated constants (per-column coefficient rows, per-supergroup
    column iotas) ship as single rows and are partition-broadcast on
    device, so per-core transfer is ~0.5MB.
  - One SPMD call; each core emits its partial window histogram; the host
    sums 8 small windows into the 400x400 output.

Self-contained: hardcodes H=W=2048, 8 cores.
"""
import hashlib
import math
import os
import time as _time

import numpy as np

import jax

# Persistent compilation cache: lets warm calls (and fresh processes on the
# same machine) skip the client-side BIR->NEFF recompile entirely.
try:
    jax.config.update("jax_compilation_cache_dir",
                      os.environ.get("BASS_JAX_CACHE_DIR", "/tmp/bass_jax_cache"))
    jax.config.update("jax_persistent_cache_min_compile_time_secs", 0.0)
    jax.config.update("jax_persistent_cache_min_entry_size_bytes", 0)
except Exception:
    pass

import concourse.bass as bass
import concourse.bacc as bacc
import concourse.mybir as mybir
import concourse.tile as tile
from concourse.bass_interp import get_hw_module
from concourse.bass_utils import run_bass_kernel_spmd

# In-process memo of the HLO->NEFF compile hook (pure function of the HLO
# bytes) as insurance for when the persistent cache misses.
try:
    import libneuronxla
    from concourse import bass2jax as _b2j
    _b2j.install_neuronx_cc_hook()
    if not getattr(libneuronxla, "_bass_memo_cc", None):
        _inner_cc = libneuronxla.neuronx_cc
        _cc_memo = {}

        def _memo_cc(code, code_format, platform_version, file_prefix):
            key = (hashlib.sha256(code).digest(), bytes(code_format),
                   str(platform_version))
            if key not in _cc_memo:
                _cc_memo[key] = _inner_cc(code, code_format, platform_version,
                                          file_prefix)
            return _cc_memo[key]

        libneuronxla.neuronx_cc = _memo_cc
        libneuronxla._bass_memo_cc = True
        _b2j.install_neuronx_cc_hook = lambda: None
except Exception:
    pass

# ---------------- problem constants (from reference.py) ----------------
H = W = 2048
N_CORES = 8
NEAR_TH = np.float32(0.1)
FAR_TH = np.float32(4.0)
H_MIN = np.float32(0.0)
H_MAX = np.float32(1.0)
CAMERA_HEIGHT = np.float32(0.0)
CELLS = int(math.ceil(40.0 / 0.1)) + 1   # 401
M = CELLS - 1                            # 400
SHIFT = math.floor(CELLS / 2.0)          # 200
MIN_PTS = 10

FX = np.float32(W / 2.0)
FY = np.float32(H / 2.0)
CX = int(FX) - 1
CY = int(FY) - 1

MAGIC = np.float32(1.5 * 2**23)          # fp32 round-to-nearest-int trick
CLAMP = 4.25                             # quantization clamp (> FAR_TH)
QLEV = 65535.0

# set by test harness for profiling; kernel() stores wall times here
TRACE = False
LAST_EXEC_NS = {}
P = 128                                  # partitions
ROW_TILES = H // P                       # 16
F32 = mybir.dt.float32
F16 = mybir.dt.float16
U16 = mybir.dt.uint16

_dt = np.float32


def _sxv():
    return ((np.arange(W, dtype=np.float64) - CX) / np.float64(FX)).astype(_dt)


def _syv():
    return ((np.arange(H, dtype=np.float64) - CY) / np.float64(FY)).astype(_dt)


# =====================================================================
# host-side interval arithmetic (plan the bin window + active tiles)
# =====================================================================
def _imul(a, b):
    c = [a[0] * b[0], a[0] * b[1], a[1] * b[0], a[1] * b[1]]
    return (min(c), max(c))


def _iadd(a, b):
    return (a[0] + b[0], a[1] + b[1])


def _coef_rows(pose, row):
    """a_i = pose[row,0]*sxv_i + pose[row,2]; b_j = pose[row,1]*syv_j"""
    p = np.asarray(pose, _dt)
    a = (p[row, 0] * _sxv() + p[row, 2]).astype(_dt)
    b = (p[row, 1] * _syv()).astype(_dt)
    k = float(p[row, 3])
    return a, b, k


def _valid_d(dlo, dhi):
    """hull of [dlo,dhi] restricted to the mask1-valid set |d| in [0.1, 4]."""
    lo, hi = None, None
    for a, b in ((-float(FAR_TH), -float(NEAR_TH)), (float(NEAR_TH), float(FAR_TH))):
        s, e = max(a, dlo), min(b, dhi)
        if s <= e:
            lo = s if lo is None else min(lo, s)
            hi = e if hi is None else max(hi, e)
    if lo is None:
        return None
    return (lo, hi)


def _plan(pose, dlo, dhi):
    d_int = _valid_d(dlo, dhi)
    if d_int is None:
        return None
    ax, bx, kx = _coef_rows(pose, 0)   # gx
    ay, by, ky = _coef_rows(pose, 1)   # gy raw
    az, bz, kz = _coef_rows(pose, 2)   # gz

    def box_for(a, b, k):
        c_int = _iadd((float(a.min()), float(a.max())),
                      (float(b.min()), float(b.max())))
        g = _iadd(_imul(d_int, c_int), (k, k))
        v = (10.0 * g[0] + SHIFT, 10.0 * g[1] + SHIFT)
        lo = int(np.floor(v[0])) - 1
        hi = int(np.ceil(v[1])) + 1
        return max(lo, -1), min(hi, M)

    rbox = box_for(az, bz, kz)
    cbox = box_for(ax, bx, kx)
    if rbox[0] > rbox[1] or cbox[0] > cbox[1]:
        return None

    u_hi = float(CAMERA_HEIGHT - ky - H_MIN)   # valid iff u_lo < w < u_hi
    u_lo = float(CAMERA_HEIGHT - ky - H_MAX)
    a_int = (float(ay.min()), float(ay.max()))
    active = []
    for t in range(ROW_TILES):
        bt = by[t * P:(t + 1) * P]
        c_int = _iadd(a_int, (float(bt.min()), float(bt.max())))
        w_int = _imul(d_int, c_int)
        if w_int[0] < u_hi and w_int[1] > u_lo:
            active.append(t)
    return dict(rbox=rbox, cbox=cbox, active=active,
                ax=ax, bx=bx, kx=kx, ay=ay, by=by, ky=ky,
                az=az, bz=bz, kz=kz, u_lo=u_lo, u_hi=u_hi)


def _pad_to(x, m):
    return ((x + m - 1) // m) * m


def _chunks(lo, hi, cap):
    out = []
    x = lo
    while x <= hi:
        wdt = min(cap, hi - x + 1)
        out.append((x, wdt))
        x += wdt
    return out


# =====================================================================
# phase 1 kernel builder
# =====================================================================
_phase1_cache = {}


def _layouts(cfg):
    """Segment layouts of the merged replicated-row inputs."""
    l16 = []
    for ri, (r0, Wr) in enumerate(cfg["r_chunks"]):
        l16.append((f"ior{ri}", Wr))
    if cfg["sgc"] is None:
        for ci, (c0, Wc) in enumerate(cfg["c_chunks"]):
            l16.append((f"ioc{ci}", Wc))
    else:
        l16.append(("iocf", (W // cfg["nb"]) * cfg["sgc"]["Wcol"]))
    l32 = []
    if cfg["ax_const"] is None:
        l32.append(("axr", W))
    if cfg["az_const"] is None:
        l32.append(("azr", W))
    if not cfg["ay_zero"]:
        l32.append(("ayr", W))
    return l16, l32


def _build_phase1(cfg):
    key = cfg["key"]
    if key in _phase1_cache:
        return _phase1_cache[key]

    n_t = cfg["n_t"]
    nb = cfg["nb"]
    r_chunks = cfg["r_chunks"]      # list of (r0, Wr)
    c_chunks = cfg["c_chunks"]      # list of (c0, Wc)
    ax_const = cfg["ax_const"]      # float or None
    az_const = cfg["az_const"]
    bx_zero = cfg["bx_zero"]
    bz_zero = cfg["bz_zero"]
    ay_zero = cfg["ay_zero"]
    kx = cfg["kx"]
    kz = cfg["kz"]
    u_lo = cfg["u_lo"]
    u_hi = cfg["u_hi"]
    sgc = cfg.get("sgc")          # per-supergroup c windows: (Wcol, bases)

    nc = bacc.Bacc("TRN2", target_bir_lowering=False, debug=False,
                   num_devices=N_CORES)
    d_dram = nc.dram_tensor("d1", [n_t * P, W], U16, kind="ExternalInput").ap()
    # per-row (partition) coefficient columns, packed [P, 4*n_t + 2]
    # (bx, by, bz, 0) per tile, then (qscale, qlo)
    b_dram = nc.dram_tensor("bcols", [P, 4 * n_t + 2], F32,
                            kind="ExternalInput").ap()
    need_ax = ax_const is None
    need_az = az_const is None
    need_ay = not ay_zero
    # merged row tensors: one f16 row (r/c iotas), one f32 row (coef rows)
    l16, l32 = _layouts(cfg)
    L16 = sum(n for _, n in l16)
    L32 = sum(n for _, n in l32)
    rowf16_dram = nc.dram_tensor("rowf16", [1, L16], F16,
                                 kind="ExternalInput").ap()
    if L32:
        rowf32_dram = nc.dram_tensor("rowf32", [1, L32], F32,
                                     kind="ExternalInput").ap()
    # merged per-partition constants: [sel_ri | m8_ri] per r chunk
    Wr0 = r_chunks[0][1]
    pm_dram = nc.dram_tensor("pm", [nb * Wr0, len(r_chunks) * (Wr0 + nb)], F32,
                             kind="ExternalInput").ap()
    win_dram = {}
    for ri, (r0, Wr) in enumerate(r_chunks):
        for ci, (c0, Wc) in enumerate(c_chunks):
            win_dram[(ri, ci)] = nc.dram_tensor(
                f"win{ri}_{ci}", [Wr, Wc], F32, kind="ExternalOutput").ap()

    A = mybir.AluOpType
    SENT_LO = float(min(r0 for r0, _ in r_chunks) - 5)
    SENT_HI = float(max(r0 + w for r0, w in r_chunks) + 4)
    PEN = 256.0  # > sentinel span (Wr+9 <= 137), 4*PEN + |SENT| < 2048 (f16 int-exact)

    with tile.TileContext(nc) as tc:
        with tc.tile_pool(name="const", bufs=1) as cpool, \
             tc.tile_pool(name="sbuf", bufs=2) as pool, \
             tc.tile_pool(name="oh", bufs=2) as ohpool, \
             tc.tile_pool(name="psum", bufs=1, space="PSUM") as psum_pool, \
             tc.tile_pool(name="psum2", bufs=2, space="PSUM") as psum2_pool:

            # ---- constants (two broadcasts cover every replicated row) ----
            r16a = cpool.tile([1, L16], F16, tag="r16a")
            nc.sync.dma_start(out=r16a, in_=rowf16_dram)
            r16 = cpool.tile([P, L16], F16, tag="r16")
            nc.gpsimd.partition_broadcast(r16, r16a)
            if L32:
                r32a = cpool.tile([1, L32], F32, tag="r32a")
                nc.sync.dma_start(out=r32a, in_=rowf32_dram)
                r32 = cpool.tile([P, L32], F32, tag="r32")
                nc.gpsimd.partition_broadcast(r32, r32a)
            seg16 = {}
            off = 0
            for name, ln in l16:
                seg16[name] = r16[:, off:off + ln]
                off += ln
            seg32 = {}
            off = 0
            for name, ln in l32:
                seg32[name] = r32[:, off:off + ln]
                off += ln
            ior = {ri: seg16[f"ior{ri}"] for ri, _ in enumerate(r_chunks)}
            if sgc is None:
                ioc = {ci: seg16[f"ioc{ci}"] for ci, _ in enumerate(c_chunks)}
            else:
                iocf = seg16["iocf"]
                zlh = cpool.tile([P, nb * r_chunks[0][1]], F16, tag="zlh")
                nc.vector.memset(zlh, 0.0)
                zrh = cpool.tile([P, nb * c_chunks[0][1]], F16, tag="zrh")
                nc.vector.memset(zrh, 0.0)
            if need_ax:
                ax_t = seg32["axr"]
            if need_az:
                az_t = seg32["azr"]
            if need_ay:
                ay_t = seg32["ayr"]
            pm = cpool.tile([nb * Wr0, len(r_chunks) * (Wr0 + nb)], F32,
                            tag="pm")
            nc.sync.dma_start(out=pm, in_=pm_dram)
            sel = {}
            m8 = {}
            for ri, (r0, Wr) in enumerate(r_chunks):
                base = ri * (Wr0 + nb)
                sel[ri] = pm[:, base:base + Wr]
                m8[ri] = pm[:, base + Wr:base + Wr + nb]
            bcols = cpool.tile([P, 4 * n_t + 2], F32, tag="bcols")
            nc.sync.dma_start(out=bcols, in_=b_dram)
            qs_ap = bcols[:, 4 * n_t + 0:4 * n_t + 1]
            ql_ap = bcols[:, 4 * n_t + 1:4 * n_t + 2]

            psum = {}
            for ri, (r0, Wr) in enumerate(r_chunks):
                for ci, (c0, Wc) in enumerate(c_chunks):
                    psum[(ri, ci)] = psum_pool.tile([nb * Wr, nb * Wc], F32,
                                                    tag=f"ps{ri}_{ci}",
                                                    name=f"ps{ri}_{ci}")

            n_super = W // nb
            if sgc is not None:
                for ri, (r0, Wr) in enumerate(r_chunks):
                    nc.tensor.matmul(psum[(ri, 0)], zlh, zrh,
                                     start=True, stop=False)
            CH = 1024                     # column chunk for pipelining
            n_cc = W // CH
            sg_per_cc = CH // nb
            for t in range(n_t):
                q = pool.tile([P, W], U16, tag="q")
                nc.sync.dma_start(out=q, in_=d_dram[t * P:(t + 1) * P, :])
                # dequant: d = qscale*q + qlo
                d = pool.tile([P, W], F32, tag="d")
                nc.scalar.activation(out=d, in_=q,
                                     func=mybir.ActivationFunctionType.Copy,
                                     bias=0.0, scale=qs_ap)
                nc.vector.tensor_scalar(out=d, in0=d, scalar1=ql_ap,
                                        scalar2=None, op0=A.add)
                bx_ap = bcols[:, 4 * t + 0:4 * t + 1]
                by_ap = bcols[:, 4 * t + 1:4 * t + 2]
                bz_ap = bcols[:, 4 * t + 2:4 * t + 3]

                for cc in range(n_cc):
                    csl = slice(cc * CH, (cc + 1) * CH)
                    dC = d[:, csl]

                    # ---- c index ----
                    vc = pool.tile([P, CH], F32, tag="vc")
                    if ax_const is None:
                        tC = pool.tile([P, CH], F32, tag="tC")
                        nc.vector.tensor_tensor(out=tC, in0=dC,
                                                in1=ax_t[:, csl], op=A.mult)
                        if not bx_zero:
                            nc.vector.scalar_tensor_tensor(
                                out=tC, in0=dC, scalar=bx_ap, in1=tC,
                                op0=A.mult, op1=A.add)
                        nc.vector.tensor_scalar(
                            out=vc, in0=tC, scalar1=10.0,
                            scalar2=float(SHIFT + 10.0 * kx),
                            op0=A.mult, op1=A.add)
                    else:
                        if not bx_zero:
                            tC = pool.tile([P, CH], F32, tag="tC")
                            nc.vector.tensor_scalar(out=tC, in0=dC, scalar1=bx_ap,
                                                    scalar2=None, op0=A.mult)
                            nc.vector.scalar_tensor_tensor(
                                out=tC, in0=dC, scalar=float(ax_const), in1=tC,
                                op0=A.mult, op1=A.add)
                            nc.vector.tensor_scalar(
                                out=vc, in0=tC, scalar1=10.0,
                                scalar2=float(SHIFT + 10.0 * kx),
                                op0=A.mult, op1=A.add)
                        else:
                            nc.vector.tensor_scalar(
                                out=vc, in0=dC, scalar1=float(10.0 * ax_const),
                                scalar2=float(SHIFT + 10.0 * kx),
                                op0=A.mult, op1=A.add)
                    vcM = pool.tile([P, CH], F32, tag="vcM")
                    nc.scalar.activation(out=vcM, in_=vc,
                                         func=mybir.ActivationFunctionType.Copy,
                                         bias=float(MAGIC))
                    vc16 = pool.tile([P, CH], F16, tag="vc16")
                    nc.scalar.activation(out=vc16, in_=vcM,
                                         func=mybir.ActivationFunctionType.Copy,
                                         bias=float(-MAGIC))

                    # ---- r index ----
                    vr = pool.tile([P, CH], F32, tag="vr")
                    if az_const is None:
                        tZ = pool.tile([P, CH], F32, tag="tZ")
                        nc.vector.tensor_tensor(out=tZ, in0=dC,
                                                in1=az_t[:, csl], op=A.mult)
                        if not bz_zero:
                            nc.vector.scalar_tensor_tensor(
                                out=tZ, in0=dC, scalar=bz_ap, in1=tZ,
                                op0=A.mult, op1=A.add)
                        nc.vector.tensor_scalar(
                            out=vr, in0=tZ, scalar1=10.0,
                            scalar2=float(SHIFT + 10.0 * kz),
                            op0=A.mult, op1=A.add)
                    else:
                        if not bz_zero:
                            tZ = pool.tile([P, CH], F32, tag="tZ")
                            nc.vector.tensor_scalar(out=tZ, in0=dC, scalar1=bz_ap,
                                                    scalar2=None, op0=A.mult)
                            nc.vector.scalar_tensor_tensor(
                                out=tZ, in0=dC, scalar=float(az_const), in1=tZ,
                                op0=A.mult, op1=A.add)
                            nc.vector.tensor_scalar(
                                out=vr, in0=tZ, scalar1=10.0,
                                scalar2=float(SHIFT + 10.0 * kz),
                                op0=A.mult, op1=A.add)
                        else:
                            nc.vector.tensor_scalar(
                                out=vr, in0=dC, scalar1=float(10.0 * az_const),
                                scalar2=float(SHIFT + 10.0 * kz),
                                op0=A.mult, op1=A.add)
                    vrM = pool.tile([P, CH], F32, tag="vrM")
                    nc.scalar.activation(out=vrM, in_=vr,
                                         func=mybir.ActivationFunctionType.Copy,
                                         bias=float(MAGIC))
                    vr16 = pool.tile([P, CH], F16, tag="vr16")
                    nc.scalar.activation(out=vr16, in_=vrM,
                                         func=mybir.ActivationFunctionType.Copy,
                                         bias=float(-MAGIC))
                    # clamp to sentinels FIRST, then add penalties (PEN >
                    # sentinel span) -- keeps every value f16-int-exact and
                    # guarantees masked points never collide with the window.
                    nc.vector.tensor_scalar(out=vr16, in0=vr16, scalar1=SENT_HI,
                                            scalar2=SENT_LO, op0=A.min, op1=A.max)

                    # ---- masks -> penalties on vr16 ----
                    wY = pool.tile([P, CH], F32, tag="wY")
                    if need_ay:
                        nc.vector.tensor_tensor(out=wY, in0=dC,
                                                in1=ay_t[:, csl], op=A.mult)
                        nc.vector.scalar_tensor_tensor(
                            out=wY, in0=dC, scalar=by_ap, in1=wY,
                            op0=A.mult, op1=A.add)
                    else:
                        nc.scalar.activation(out=wY, in_=dC,
                                             func=mybir.ActivationFunctionType.Copy,
                                             bias=0.0, scale=by_ap)
                    vio = pool.tile([P, CH], F16, tag="vio")
                    ad = pool.tile([P, CH], F32, tag="ad")
                    nc.scalar.activation(out=ad, in_=dC,
                                         func=mybir.ActivationFunctionType.Abs)
                    for src_t, thr, cmp in ((wY, float(u_hi), A.is_ge),
                                            (wY, float(u_lo), A.is_le),
                                            (ad, float(NEAR_TH), A.is_lt),
                                            (ad, float(FAR_TH), A.is_ge)):
                        nc.vector.tensor_scalar(out=vio, in0=src_t, scalar1=thr,
                                                scalar2=PEN, op0=cmp, op1=A.mult)
                        nc.vector.tensor_tensor(out=vr16, in0=vr16, in1=vio,
                                                op=A.add)

                    # ---- one-hot + matmul accumulate ----
                    G = 32
                    n_groups = sg_per_cc // G
                    for g2 in range(n_groups):
                        sl = slice(g2 * G * nb, (g2 + 1) * G * nb)
                        lhsT = {}
                        for ri, (r0, Wr) in enumerate(r_chunks):
                            lt = ohpool.tile([P, G * nb * Wr], F16,
                                             tag=f"lh{ri}", name=f"lh{ri}")
                            nc.vector.tensor_tensor(
                                out=lt.rearrange("p (n w) -> p n w", w=Wr),
                                in0=vr16[:, sl][:, :, None].broadcast_to([P, G * nb, Wr]),
                                in1=ior[ri][:, None, :].broadcast_to([P, G * nb, Wr]),
                                op=A.is_equal)
                            lhsT[ri] = lt
                        rhs = {}
                        if sgc is None:
                            for ci, (c0, Wc) in enumerate(c_chunks):
                                rh = ohpool.tile([P, G * nb * Wc], F16,
                                                 tag=f"rh{ci}", name=f"rh{ci}")
                                nc.vector.tensor_tensor(
                                    out=rh.rearrange("p (n w) -> p n w", w=Wc),
                                    in0=vc16[:, sl][:, :, None].broadcast_to([P, G * nb, Wc]),
                                    in1=ioc[ci][:, None, :].broadcast_to([P, G * nb, Wc]),
                                    op=A.is_equal)
                                rhs[ci] = rh
                        else:
                            WCOL = sgc["Wcol"]
                            s_base = cc * sg_per_cc + g2 * G
                            rh = ohpool.tile([P, G * nb * WCOL], F16,
                                             tag="rh0", name="rh0")
                            vcv = vc16[:, sl].rearrange("p (g n) -> p g n", g=G)
                            iov = iocf[:, s_base * WCOL:(s_base + G) * WCOL] \
                                .rearrange("p (g w) -> p g w", g=G)
                            nc.vector.tensor_tensor(
                                out=rh.rearrange("p (g n w) -> p g n w", g=G, w=WCOL),
                                in0=vcv[:, :, :, None].broadcast_to([P, G, nb, WCOL]),
                                in1=iov[:, :, None, :].broadcast_to([P, G, nb, WCOL]),
                                op=A.is_equal)
                            rhs[0] = rh
                        for k in range(G):
                            s = cc * sg_per_cc + g2 * G + k
                            last = (t == n_t - 1) and (s == n_super - 1)
                            for ci, (c0, Wc) in enumerate(c_chunks):
                                for ri, (r0, Wr) in enumerate(r_chunks):
                                    if sgc is None:
                                        nc.tensor.matmul(
                                            psum[(ri, ci)],
                                            lhsT[ri][:, k * nb * Wr:(k + 1) * nb * Wr],
                                            rhs[ci][:, k * nb * Wc:(k + 1) * nb * Wc],
                                            start=(s == 0 and t == 0),
                                            stop=last)
                                    else:
                                        WCOL = sgc["Wcol"]
                                        o_s = sgc["bases"][s] - c0
                                        out_ap = psum[(ri, ci)].rearrange(
                                            "m (n q) -> m n q", q=Wc)[:, :, o_s:o_s + WCOL]
                                        nc.tensor.matmul(
                                            out_ap,
                                            lhsT[ri][:, k * nb * Wr:(k + 1) * nb * Wr],
                                            rhs[ci][:, k * nb * WCOL:(k + 1) * nb * WCOL],
                                            start=False,
                                            stop=last)
            # ---- extract: cross-block fold ----
            for ri, (r0, Wr) in enumerate(r_chunks):
                for ci, (c0, Wc) in enumerate(c_chunks):
                    psb = pool.tile([nb * Wr, nb * Wc], F32, tag="psb")
                    nc.vector.tensor_tensor(
                        out=psb.rearrange("p (n w) -> p n w", n=nb),
                        in0=psum[(ri, ci)].rearrange("p (n w) -> p n w", n=nb),
                        in1=m8[ri][:, :, None].broadcast_to([nb * Wr, nb, Wc]),
                        op=A.mult)
                    ps2 = psum2_pool.tile([Wr, nb * Wc], F32, tag="ps2")
                    nc.tensor.matmul(ps2, sel[ri], psb, start=True, stop=True)
                    o2 = pool.tile([Wr, nb * Wc], F32, tag="o2")
                    nc.vector.tensor_copy(out=o2, in_=ps2)
                    acc = pool.tile([Wr, Wc], F32, tag="acc")
                    nc.vector.tensor_copy(out=acc, in_=o2[:, 0:Wc])
                    for b in range(1, nb):
                        nc.vector.tensor_tensor(out=acc, in0=acc,
                                                in1=o2[:, b * Wc:(b + 1) * Wc],
                                                op=A.add)
                    nc.sync.dma_start(out=win_dram[(ri, ci)], in_=acc)

    nc.compile()
    nc.m = get_hw_module(nc.m)
    _phase1_cache[key] = nc
    return nc


# =====================================================================
# host fallback (exact reference replication, used for gate corner cases)
# =====================================================================
def _host_reference(depth, pose):
    d = np.asarray(depth, _dt)
    pose = np.asarray(pose, _dt)
    sx = _sxv()
    sy = _syv()
    px = d * sx[None, :]
    py = d * sy[:, None]
    pz = d
    mask1 = (np.abs(pz) < FAR_TH) & (np.abs(pz) >= NEAR_TH)
    ones = np.ones_like(d)
    gx = pose[0, 0] * px + pose[0, 1] * py + pose[0, 2] * pz + pose[0, 3] * ones
    gy = pose[1, 0] * px + pose[1, 1] * py + pose[1, 2] * pz + pose[1, 3] * ones
    gz = pose[2, 0] * px + pose[2, 1] * py + pose[2, 2] * pz + pose[2, 3] * ones
    gy = -gy + CAMERA_HEIGHT
    mask2 = mask1 & (gy > H_MIN) & (gy < H_MAX)
    r = np.round(gz / _dt(0.1) + _dt(SHIFT)).astype(np.int64)
    c = np.round(gx / _dt(0.1) + _dt(SHIFT)).astype(np.int64)
    inb = (r >= 0) & (r < M) & (c >= 0) & (c < M)
    valid = mask2 & inb
    flat = np.where(valid, r * M + c, 0)
    hist = np.bincount(flat.ravel(), weights=valid.ravel().astype(np.float64),
                       minlength=M * M).astype(_dt).reshape(M, M)
    n1 = int(mask1.sum())
    n2 = int(mask2.sum())
    ok = (n1 >= 20) and (n2 > MIN_PTS)
    return hist if ok else np.zeros((M, M), _dt)


# =====================================================================
# main entry
# =====================================================================
_static_cache = {}


def _make_cfg(plan, dlo, dhi):
    r_lo, r_hi = plan["rbox"]
    c_lo, c_hi = plan["cbox"]
    boxw_r = r_hi - r_lo + 1
    boxw_c = c_hi - c_lo + 1

    Wr_u = min(128, _pad_to(boxw_r, 2))
    nb = 1
    while nb < 8 and 2 * nb * Wr_u <= P:
        nb *= 2
    r_chunks = _chunks(r_lo, r_hi, Wr_u)
    r_chunks = [(r0, Wr_u) for (r0, w) in r_chunks]
    c_cap = (512 // nb) & ~1
    c_chunks = _chunks(c_lo, c_hi, c_cap)
    c_chunks = [(c0, _pad_to(w, 2)) for (c0, w) in c_chunks]
    assert len(r_chunks) * len(c_chunks) <= 6, "window too large for PSUM"

    sgc = None
    if len(c_chunks) == 1:
        n_super_all = W // nb
        ax_v, bx_v = plan["ax"], plan["bx"]
        kx_v = plan["kx"]
        bxa = np.concatenate([bx_v[t * P:(t + 1) * P] for t in plan["active"]]) \
            if plan["active"] else bx_v
        bx_int = (float(bxa.min()), float(bxa.max()))
        d_int = _valid_d(dlo, dhi)
        bases = []
        tops = []
        for s in range(n_super_all):
            ag = ax_v[s * nb:(s + 1) * nb]
            ci_ = _iadd((float(ag.min()), float(ag.max())), bx_int)
            g = _iadd(_imul(d_int, ci_), (kx_v, kx_v))
            v = (10.0 * g[0] + SHIFT, 10.0 * g[1] + SHIFT)
            bases.append(max(int(np.floor(v[0])) - 1, c_lo))
            tops.append(min(int(np.ceil(v[1])) + 1, c_lo + c_chunks[0][1] - 1))
        Wcol = _pad_to(max(t - b + 1 for b, t in zip(bases, tops)), 2)
        bases = [min(b, c_lo + c_chunks[0][1] - Wcol) for b in bases]
        if Wcol + 4 < c_chunks[0][1]:
            sgc = dict(Wcol=Wcol, bases=tuple(bases))

    active = plan["active"]
    n_t = (len(active) + N_CORES - 1) // N_CORES

    ax, bx = plan["ax"], plan["bx"]
    ay, by = plan["ay"], plan["by"]
    az, bz = plan["az"], plan["bz"]
    ax_const = float(ax[0]) if np.all(ax == ax[0]) else None
    az_const = float(az[0]) if np.all(az == az[0]) else None
    bx_zero = bool(np.all(bx == 0))
    bz_zero = bool(np.all(bz == 0))
    ay_zero = bool(np.all(ay == 0))

    cfg = dict(
        key=(n_t, nb, tuple(r_chunks), tuple(c_chunks),
             ax_const, az_const, bx_zero, bz_zero, ay_zero,
             plan["kx"], plan["kz"], plan["u_lo"], plan["u_hi"],
             (sgc["Wcol"], sgc["bases"]) if sgc else None),
        n_t=n_t, nb=nb, r_chunks=r_chunks, c_chunks=c_chunks,
        ax_const=ax_const, az_const=az_const,
        bx_zero=bx_zero, bz_zero=bz_zero, ay_zero=ay_zero,
        kx=plan["kx"], kz=plan["kz"], u_lo=plan["u_lo"], u_hi=plan["u_hi"],
        sgc=sgc)
    return cfg


_opt_cache = {}   # pose -> (plan, cfg) for the fixed [0,1] grid
_qbuf_cache = {}  # shape -> reusable u16 quantization buffer


def _is_contig(active, n_t):
    return (active == list(range(active[0], active[0] + len(active)))
            and len(active) == n_t * N_CORES)


def _buf(key, shape, dtype):
    b = _qbuf_cache.get(key)
    if b is None or b.shape != shape:
        b = _qbuf_cache[key] = np.empty(shape, dtype)
    return b


def kernel(depth, pose):
    t_start = _time.perf_counter()
    depth = np.asarray(depth, _dt)
    pose = np.asarray(pose, _dt)
    assert depth.shape == (H, W)

    # ---- fast path: fixed [0,1] grid; plan/cfg depend only on the pose ----
    # Quantize the (pose-determined) active block first and validate its
    # range on the quantized temp; fall back to the exact path if out of
    # range. Inactive rows never contribute, so their values don't matter.
    qs1 = 1.0 / QLEV
    qall = None
    pk = pose.tobytes()
    if _opt_cache.get("pose") != pk:
        p_o = _plan(pose, 0.0 - qs1, 1.0 + qs1)
        c_o = _make_cfg(p_o, 0.0 - qs1, 1.0 + qs1) \
            if (p_o is not None and p_o["active"]) else None
        _opt_cache.clear()
        _opt_cache.update(pose=pk, plan=p_o, cfg=c_o)
    plan, cfg = _opt_cache["plan"], _opt_cache["cfg"]
    fast = cfg is not None and _is_contig(plan["active"], cfg["n_t"])
    if fast:
        a0 = plan["active"][0]
        n_act = len(plan["active"])
        rows = depth[a0 * P:(a0 + n_act) * P, :]
        tmp = _buf("qtmp", rows.shape, np.float32)
        np.multiply(rows, _dt(QLEV), out=tmp)
        tmp += _dt(0.5)
        tmn = float(tmp.min())
        tmx = float(tmp.max())
        if tmn >= 0.5 and tmx <= QLEV + 0.5:
            qall = _buf("qall", rows.shape, np.uint16)
            np.copyto(qall, tmp, casting='unsafe')
            q_lo, q_hi = 0.0, 1.0
            q_scale = qs1
            dmin = (tmn - 0.5) * qs1
            dmax = (tmx - 0.5) * qs1
    if qall is None:
        # ---- exact path ----
        dmin = float(depth.min())
        dmax = float(depth.max())
        # quantization domain, snapped to a coarse grid for config stability
        q_lo = max(math.floor(dmin * 16.0) / 16.0, -CLAMP)
        q_hi = min(math.ceil(dmax * 16.0) / 16.0, CLAMP)
        if q_hi <= q_lo:
            q_hi = q_lo + 1.0 / 16.0
        q_scale = (q_hi - q_lo) / QLEV
        # plan over the clamped range, padded by one quantization step
        dlo = max(dmin, -float(FAR_TH)) - q_scale
        dhi = min(dmax, float(FAR_TH)) + q_scale
        plan = _plan(pose, dlo, dhi)
        if plan is None or not plan["active"]:
            return _host_reference(depth, pose)
        cfg = _make_cfg(plan, dlo, dhi)

    # Padded slab rows (when active tiles don't divide evenly) carry q=0,
    # which dequantizes to d=q_lo with by=0. Verify such rows are always
    # masked (near/far or height band); else fall back to the exact host path.
    n_fill = cfg["n_t"] * N_CORES - len(plan["active"])
    if n_fill > 0:
        d_f = q_lo
        safe = abs(d_f) < float(NEAR_TH) or abs(d_f) >= float(FAR_TH)
        if not safe:
            ay_v = plan["ay"]
            w_lo = min(d_f * float(ay_v.min()), d_f * float(ay_v.max()))
            w_hi = max(d_f * float(ay_v.min()), d_f * float(ay_v.max()))
            safe = (w_hi <= plan["u_lo"]) or (w_lo >= plan["u_hi"])
        if not safe:
            return _host_reference(depth, pose)

    nc = _build_phase1(cfg)

    r_chunks = cfg["r_chunks"]
    c_chunks = cfg["c_chunks"]
    nb = cfg["nb"]
    n_t = cfg["n_t"]
    sgc = cfg["sgc"]
    active = plan["active"]
    ax, bx = plan["ax"], plan["bx"]
    ay, by = plan["ay"], plan["by"]
    az, bz = plan["az"], plan["bz"]
    ax_const = cfg["ax_const"]
    az_const = cfg["az_const"]
    ay_zero = cfg["ay_zero"]

    # ---- shared aux inputs (static per cfg+pose+quant grid: cached) ----
    static_key = (cfg["key"], pose.tobytes(), q_lo, q_scale)
    cached = _static_cache.get("k") == static_key
    if not cached:
        l16, l32 = _layouts(cfg)
        seg16 = {}
        for ri, (r0, Wr) in enumerate(r_chunks):
            seg16[f"ior{ri}"] = (r0 + np.arange(Wr)).astype(np.float16)
        if sgc is None:
            for ci, (c0, Wc) in enumerate(c_chunks):
                seg16[f"ioc{ci}"] = (c0 + np.arange(Wc)).astype(np.float16)
        else:
            Wcol = sgc["Wcol"]
            vals = (np.asarray(sgc["bases"], np.float32)[:, None]
                    + np.arange(Wcol, dtype=np.float32)[None, :]).astype(np.float16)
            seg16["iocf"] = vals.reshape(-1)
        seg32 = {"axr": ax, "azr": az, "ayr": ay}
        aux_inputs = {}
        aux_inputs["rowf16"] = np.concatenate(
            [seg16[name] for name, _ in l16])[None, :]
        if l32:
            aux_inputs["rowf32"] = np.concatenate(
                [seg32[name] for name, _ in l32])[None, :]
        Wr0 = r_chunks[0][1]
        pm = np.zeros((nb * Wr0, len(r_chunks) * (Wr0 + nb)), _dt)
        pidx = np.arange(nb * Wr0)
        for ri, (r0, Wr) in enumerate(r_chunks):
            base = ri * (Wr0 + nb)
            pm[pidx, base + pidx % Wr] = 1.0
            pm[pidx, base + Wr + pidx // Wr] = 1.0
        aux_inputs["pm"] = pm
        bcols_percore = []
        for g in range(N_CORES):
            tiles = active[g::N_CORES]
            bcols = np.zeros((P, 4 * n_t + 2), _dt)
            for k, t in enumerate(tiles):
                bcols[:, 4 * k + 0] = bx[t * P:(t + 1) * P]
                bcols[:, 4 * k + 1] = by[t * P:(t + 1) * P]
                bcols[:, 4 * k + 2] = bz[t * P:(t + 1) * P]
            bcols[:, 4 * n_t + 0] = q_scale
            bcols[:, 4 * n_t + 1] = q_lo
            bcols_percore.append(bcols)
        _static_cache.clear()
        _static_cache.update(k=static_key, aux=aux_inputs, bcols=bcols_percore)
    aux_inputs = _static_cache["aux"]
    bcols_percore = _static_cache["bcols"]

    # ---- per-core inputs: quantized depth slabs ----
    inv_scale = _dt(1.0 / q_scale)
    # round-half-up via +0.5 and truncation; values are >= 0 after -q_lo
    q_off = _dt(0.5 - q_lo / q_scale)
    need_clip = dmin < -CLAMP or dmax > CLAMP
    contig = qall is not None or _is_contig(active, n_t)
    if contig and qall is None:
        # single vectorized quantization over the contiguous active block,
        # into a reused buffer
        rows = depth[active[0] * P:(active[0] + len(active)) * P, :]
        if need_clip:
            rows = np.clip(rows, -CLAMP, CLAMP)
        qall = _buf("qall", rows.shape, np.uint16)
        np.copyto(qall, rows * inv_scale + q_off, casting='unsafe')
    in_maps = []
    for g in range(N_CORES):
        tiles = active[g::N_CORES]
        if contig:
            # tiles are active[0]+g, active[0]+g+8, ... -> strided view rows
            dslab = np.concatenate(
                [qall[(t - active[0]) * P:(t - active[0] + 1) * P, :]
                 for t in tiles], axis=0) if n_t > 1 else \
                qall[(tiles[0] - active[0]) * P:(tiles[0] - active[0] + 1) * P, :]
        else:
            dslab = np.zeros((n_t * P, W), np.uint16)
            for k, t in enumerate(tiles):
                rows = depth[t * P:(t + 1) * P, :]
                if need_clip:
                    rows = np.clip(rows, -CLAMP, CLAMP)
                dslab[k * P:(k + 1) * P, :] = \
                    (rows * inv_scale + q_off).astype(np.uint16)
        im = {"d1": dslab, "bcols": bcols_percore[g]}
        im.update(aux_inputs)
        in_maps.append(im)

    LAST_EXEC_NS["prep_wall"] = int((_time.perf_counter() - t_start) * 1e9)
    _t0 = _time.perf_counter()
    res = run_bass_kernel_spmd(nc, in_maps, core_ids=list(range(N_CORES)),
                               trace=TRACE)
    LAST_EXEC_NS["phase1_wall"] = int((_time.perf_counter() - _t0) * 1e9)
    if TRACE:
        LAST_EXEC_NS["phase1"] = res.exec_time_ns

    hist = np.zeros((M, M), _dt)
    for ri, (r0, Wr) in enumerate(r_chunks):
        for ci, (c0, Wc) in enumerate(c_chunks):
            tot = np.zeros((Wr, Wc), np.float64)
            for r in res.results:
                tot += r[f"win{ri}_{ci}"]
            rs = max(r0, 0)
            re = min(r0 + Wr, M)
            cs = max(c0, 0)
            ce = min(c0 + Wc, M)
            if rs < re and cs < ce:
                hist[rs:re, cs:ce] = tot[rs - r0:re - r0, cs - c0:ce - c0]

    if hist.sum() < 4096:
        return _host_reference(depth, pose)
    return hist.astype(_dt)


if __name__ == "__main__":
    rng = np.random.default_rng(0)
    d = rng.random((H, W), _dt)
    p = np.eye(4, dtype=_dt)
    out = kernel(d, p)
    print("sum", out.sum(), "nonzero", (out > 0).sum())
